# revision 21
# baseline (speedup 1.0000x reference)
"""BREWA (bit-witness) attention on 8 TRN2 NeuronCores.

Sharding: core c = (batch b, head-group g) with b = c // 2, g = c % 2.
Each core computes its batch's attention for 8 of the 16 heads plus the
partial output projection over those heads' Wo columns; the host sums the
two partial projections per batch (the "all-reduce" is 2-way, done on CPU).

Key structural trick: Q and K are consumed ONLY through the per-head bit
encoders tanh((x@Wq^T)_h @ W_enc[h]), so the two linear maps are folded
on the host into Wqe[h] = Wq_h^T @ W_enc[h] ([1024 -> 32] per head) —
the full-width QK projections never exist on device (-35us PE, -224
matmuls, and one less bf16 rounding).

Per-core dataflow (all matmuls bf16, fp32 PSUM accumulate):
  xT[b] (host-transposed, bf16)
    --PE (folded Wqe/Wke, K=1024)--> enc psum --ACT tanh--> q_encT,k_encT
        [128, 2048] per quad: 4 heads x 32 bits on partitions, seq free
    --PE--> V [2048,520] (seq on partitions; 65 cols/head: 64 V + ones)
  per (head-pair p, q-tile 512, k-tile 128):
    ST[k,q] via 2 row-tiled K=32 matmuls -> st psum [128, 1024]
    exp(ST/sqrt32) on ACT -> SBUF bf16   (softmax w/o max-sub: |scores|<=5.66)
    att[r] += V_aug[kt,h].T @ expST      (psum [65,512]; row 64 = sum_k exp = Z)
  normalize: DVE reciprocal(Z) -> GPSIMD partition_broadcast -> DVE mul -> c_T
  y = c_T.T @ WoT_g  (per-core partial, bf16 out; host upcasts + sums)

Scheduling (metronome + fill queue): DMA is split and ordered
(interleaved wke/xt-nt0 k-pairs, wqe, wv, xt-nt1..3, wo) so the first
encoder matmul starts ~1us in. Each unit (head-pair, qt) emits a tight
ST->exp stream — the metronome that keeps ACT saturated; between kt
quarters it drains NFILL pieces from a deque: the previous unit's
attV+normalize (queue front, one-unit lag decouples attV from exp),
then deferred encoder slices and the out-proj chase (back). Unit (0,0)
interleaves the quad-0 k-enc slices directly (hard dep of its kt
sweep). PSUM: st 2x[128,1024] + att 2x[65,512] + bank 2x[128,512] = 8
banks. The tail out-proj is staged (ready contractions ahead of the
ct-DMA-gated last k) with its y DMAs on the then-idle ACT queue.

fp8e4 DoubleRow for the scores matmul was tried and removed:
microbenchmarks measured DR at parity/slower than bf16 on real silicon
(cost model claims 2x), and it costs 3.5x the rel-err margin (archived
in kernel_v3.py).
"""

import numpy as np
import ml_dtypes

import concourse.bacc as bacc
import concourse.bass as bass
import concourse.mybir as mybir
import concourse.tile as tile
from concourse.bass_utils import run_bass_kernel_spmd

B, N, D = 4, 2048, 1024
H, HD, MB = 16, 64, 32
NCORES = 8
HPG = 8              # heads per group (per core)
GD = HPG * HD        # 512 head dims per group
SCALE = float(1.0 / np.sqrt(MB))

bf16 = mybir.dt.bfloat16
f32 = mybir.dt.float32
BF = ml_dtypes.bfloat16
AF = mybir.ActivationFunctionType

KT_X = D // 128      # 8 contraction tiles over d_model
NT = N // 512        # 4 column tiles of 512 over sequence
MT_QK = GD // 128    # 4 partition tiles of QT/KT
NT128 = N // 128     # 16 row tiles of 128 over sequence
KT_C = GD // 128     # 4 contraction tiles over group head dims

TRACE = False        # set by test.py for profiling runs
TRACE_KW = {}
LAST_RESULTS = None
PHASE_LIMIT = "full"  # "qkv" | "attn" | "full" — for sim phase ablation
NFILL = 2            # fill pieces drained per metronome quarter


def build(reps=1):
    nc = bacc.Bacc("TRN2", target_bir_lowering=False, debug=False,
                   num_devices=NCORES)
    EB = HPG * MB        # 256 encoder bits per core (8 heads x 32)
    xt = nc.dram_tensor("xt", [D, N], bf16, kind="ExternalInput").ap()
    # folded encoder weights: Wqe = Wq_h^T @ W_enc[h] per head, [1024, 256]
    # (Q/K are consumed only through the encoders, so the full-width QK
    # projections fold away entirely)
    wqe = nc.dram_tensor("wqe", [D, EB], bf16, kind="ExternalInput").ap()
    wke = nc.dram_tensor("wke", [D, EB], bf16, kind="ExternalInput").ap()
    wv = nc.dram_tensor("wv", [D, GD], bf16, kind="ExternalInput").ap()
    wo = nc.dram_tensor("wo", [GD, D], bf16, kind="ExternalInput").ap()
    y = nc.dram_tensor("y", [N, D], bf16, kind="ExternalOutput").ap()

    with tile.TileContext(nc) as tc:
        with (
            tc.tile_pool(name="xtp", bufs=KT_X) as xt_pool,
            tc.tile_pool(name="wp", bufs=3 * KT_X) as w_pool,
            tc.tile_pool(name="wop", bufs=KT_C) as wo_pool,
            tc.tile_pool(name="encp", bufs=4) as enc_pool,
            tc.tile_pool(name="vp", bufs=NT128) as v_pool,
            tc.tile_pool(name="expp", bufs=16) as exp_pool,
            tc.tile_pool(name="ctp", bufs=KT_C) as ct_pool,
            tc.tile_pool(name="smallp", bufs=4) as small_pool,
            tc.tile_pool(name="yp", bufs=3) as y_pool,
            tc.tile_pool(name="stp", bufs=2, space="PSUM") as st_pool,
            tc.tile_pool(name="attp", bufs=2, space="PSUM") as att_pool,
            tc.tile_pool(name="bankp", bufs=2, space="PSUM") as bank_pool,
        ):
          for _rep in range(reps):
            # ---- input loads: each tensor's k-slices split across BOTH
            # HWDGE queues (even k -> SP, odd k -> ACT; ACT descriptors
            # precede every ACT compute instruction in program order), in
            # consumer-priority order so both rings fill the pipe in
            # parallel at ~2x the single-queue rate ---------------------------
            def load_w(w_ap, cols, name, eng=None):
                tiles = []
                for k in range(KT_X):
                    t = w_pool.tile([128, cols], bf16, tag=f"w{cols}",
                                    name=f"{name}{k}")
                    (eng or nc.scalar).dma_start(
                        t[:], w_ap[128 * k:128 * (k + 1), :])
                    tiles.append(t)
                return tiles

            xt_sb = [xt_pool.tile([128, N], bf16, tag="xt", name=f"xt{k}")
                     for k in range(KT_X)]

            def load_xt_nt(nt):
                for k in range(KT_X):
                    nc.sync.dma_start(
                        xt_sb[k][:, 512 * nt:512 * (nt + 1)],
                        xt[128 * k:128 * (k + 1), 512 * nt:512 * (nt + 1)])

            # wke on the ACT ring, xt nt0 on the SP ring, emitted pairwise so
            # the first encoder m-tile's k-loop starts after the first pair
            # lands; the rings issue concurrently on hw
            wke_sb = []
            for k in range(KT_X):
                t = w_pool.tile([128, EB], bf16, tag="w256", name=f"wke{k}")
                nc.scalar.dma_start(t[:], wke[128 * k:128 * (k + 1), :])
                wke_sb.append(t)
                nc.sync.dma_start(
                    xt_sb[k][:, 0:512],
                    xt[128 * k:128 * (k + 1), 0:512])

            wqe_sb = load_w(wqe, EB, "wqe")
            for nt in range(1, NT):
                load_xt_nt(nt)
            wv_sb = load_w(wv, GD, "wv")
            wo_sb = []
            for k in range(KT_C):
                t = wo_pool.tile([128, D], bf16, tag="wo")
                nc.scalar.dma_start(t[:], wo[128 * k:128 * (k + 1), :])
                wo_sb.append(t)

            # ---- encoders: [128 bits (4 heads x 32), 2048 seq] per quad ----
            q_enc = [enc_pool.tile([128, N], bf16, tag="enc",
                                   name=f"qenc{d}") for d in range(2)]
            k_enc = [enc_pool.tile([128, N], bf16, tag="enc",
                                   name=f"kenc{d}") for d in range(2)]

            def enc_mtile_nt(wsb, et, qd, nt, name):
                # one folded x->bits projection: psum[bit, seq] -> tanh
                ps = bank_pool.tile([128, 512], f32, tag="bank",
                                    name=f"ps_{name}_{nt}")
                for k in range(KT_X):
                    nc.tensor.matmul(
                        ps[:],
                        wsb[k][:, 128 * qd:128 * (qd + 1)],
                        xt_sb[k][:, 512 * nt:512 * (nt + 1)],
                        start=(k == 0), stop=(k == KT_X - 1),
                        skip_group_check=True,
                    )
                nc.scalar.activation(et[:, 512 * nt:512 * (nt + 1)],
                                     ps[:], AF.Tanh)

            v_sb = [None] * NT128

            def ensure_v(nt):
                if v_sb[nt] is not None:
                    return v_sb[nt]
                t = v_pool.tile([128, HPG * 65], bf16, tag="v", name=f"v{nt}")
                ps = bank_pool.tile([128, 512], f32, tag="bank",
                                    name=f"ps_v{nt}")
                for k in range(KT_X):
                    nc.tensor.matmul(
                        ps[:],
                        xt_sb[k][:, 128 * nt:128 * (nt + 1)],
                        wv_sb[k][:],
                        start=(k == 0), stop=(k == KT_X - 1),
                        skip_group_check=True,
                    )
                vv = t[:, :].rearrange("p (h s) -> p h s", h=HPG)
                nc.vector.tensor_copy(
                    vv[:, :, 0:64],
                    ps[:, :].rearrange("p (h s) -> p h s", h=HPG),
                )
                nc.vector.memset(vv[:, :, 64:65], 1.0)
                v_sb[nt] = t
                return t

            # ---- c_T accumulator tiles: [512 head dims, 2048 seq] -----------
            ct_sb = [ct_pool.tile([128, N], bf16, tag="ct", name=f"ct{i}")
                     for i in range(KT_C)]

            def st_exp_kt(p, qt, kt):
                """ST -> exp for heads (2p, 2p+1) at one kt; returns ex."""
                qd = p // 2
                st = st_pool.tile([128, N // 2], f32, tag="st")
                for r in range(2):
                    a = 2 * (p % 2) + r
                    nc.tensor.matmul(
                        st[:, 512 * r:512 * (r + 1)],
                        k_enc[qd][32 * a:32 * (a + 1),
                                  128 * kt:128 * (kt + 1)],
                        q_enc[qd][32 * a:32 * (a + 1),
                                  512 * qt:512 * (qt + 1)],
                        start=True, stop=True,
                        tile_position=(32 * a, 0),
                        skip_group_check=True,
                    )
                ex = exp_pool.tile([128, N // 2], bf16, tag="exp")
                nc.scalar.activation(ex[:], st[:], AF.Exp, scale=SCALE)
                return ex

            def attv_kts(p, att, exs, kt_lo, kt_hi):
                # r=1 first: normalize frees att[1] first (its chain is
                # emitted first), so the first accumulate here waits on the
                # earlier-freed tile
                for kt in range(kt_lo, kt_hi):
                    for r in (1, 0):
                        h = 2 * p + r
                        nc.tensor.matmul(
                            att[r][0:65, :],
                            ensure_v(kt)[:, 65 * h:65 * h + 65],
                            exs[kt][:, 512 * r:512 * (r + 1)],
                            start=(kt == 0), stop=(kt == NT128 - 1),
                            skip_group_check=True,
                        )

            def normalize(p, qt, att):
                # pipelined across DVE/Pool: both recips issue first, then
                # both broadcasts, then both muls; the r=1 chain leads and
                # everything is lane-aligned (r=1 data at partitions 64-127)
                # so ct is written directly — no partition-shift DMA
                # NOTE: reciprocal_approx_fast passes CoreSim but returns
                # garbage on hw through this compile path (custom-DVE
                # table likely not shipped by the bass2jax/axon NEFF
                # build) — keep the exact reciprocal
                qs = slice(512 * qt, 512 * (qt + 1))
                rec1 = small_pool.tile([1, 512], f32, tag="recip")
                nc.vector.reciprocal(rec1[:], att[1][64:65, :])
                rec0 = small_pool.tile([1, 512], f32, tag="recip")
                nc.vector.reciprocal(rec0[:], att[0][64:65, :])
                bc1 = small_pool.tile([64, 512], f32, tag="bc")
                nc.gpsimd.partition_broadcast(bc1[:], rec1[:])
                bc0 = small_pool.tile([64, 512], f32, tag="bc")
                nc.gpsimd.partition_broadcast(bc0[:], rec0[:])
                tmp = small_pool.tile([64, 512], bf16, tag="tmp")
                nc.vector.tensor_mul(tmp[:], att[1][0:64, :], bc1[:])
                nc.vector.tensor_mul(
                    ct_sb[p][0:64, qs], att[0][0:64, :], bc0[:])
                # odd head's partition shift 0-63 -> 64-127: gpsimd tensor
                # copy (SBUF->SBUF, cross-partition) chained on the Pool
                # queue — much lower latency than the old SBUF-shift DMA
                nc.gpsimd.tensor_copy(ct_sb[p][64:128, qs], tmp[:])

            def new_att(p, qt):
                # full-bank tiles: r=1 uses partitions 63-127, r=0 uses 0-64
                return [att_pool.tile([128, 512], f32, tag="att",
                                      name=f"att{p}_{qt}_{r}")
                        for r in range(2)]


            def out_proj_mt(mt):
                # y rows 128*mt .. 128*(mt+1): 2 out-dim halves
                for nt2 in range(2):
                    ps = bank_pool.tile([128, 512], f32, tag="bank",
                                        name=f"ps_y{mt}_{nt2}")
                    for k in range(KT_C):
                        nc.tensor.matmul(
                            ps[:],
                            ct_sb[k][:, 128 * mt:128 * (mt + 1)],
                            wo_sb[k][:, 512 * nt2:512 * (nt2 + 1)],
                            start=(k == 0), stop=(k == KT_C - 1),
                            skip_group_check=True,
                        )
                    yt = y_pool.tile([128, 512], bf16, tag="y")
                    nc.vector.tensor_copy(yt[:], ps[:])
                    nc.sync.dma_start(
                        y[128 * mt:128 * (mt + 1),
                          512 * nt2:512 * (nt2 + 1)],
                        yt[:])

            def out_proj_qt(qt):
                for mt in range(4 * qt, 4 * qt + 4):
                    out_proj_mt(mt)

            def out_proj_qt_staged(qt):
                # tail variant: PE executes in order, so emit k=0..2 of two
                # groups before their k=3 (which waits on the last ct DMA);
                # the ready contractions fill the wait.
                slots = [(mt, nt2) for mt in range(4 * qt, 4 * qt + 4)
                         for nt2 in range(2)]
                for i in range(0, len(slots), 2):
                    pss = []
                    for mt, nt2 in slots[i:i + 2]:
                        ps = bank_pool.tile([128, 512], f32, tag="bank",
                                            name=f"ps_y{mt}_{nt2}")
                        for k in range(KT_C - 1):
                            nc.tensor.matmul(
                                ps[:],
                                ct_sb[k][:, 128 * mt:128 * (mt + 1)],
                                wo_sb[k][:, 512 * nt2:512 * (nt2 + 1)],
                                start=(k == 0), stop=False,
                                skip_group_check=True,
                            )
                        pss.append(ps)
                    for (mt, nt2), ps in zip(slots[i:i + 2], pss):
                        nc.tensor.matmul(
                            ps[:],
                            ct_sb[KT_C - 1][:, 128 * mt:128 * (mt + 1)],
                            wo_sb[KT_C - 1][:, 512 * nt2:512 * (nt2 + 1)],
                            start=False, stop=True,
                            skip_group_check=True,
                        )
                        yt = y_pool.tile([128, 512], bf16, tag="y")
                        nc.vector.tensor_copy(yt[:], ps[:])
                        # tail: alternate ACT/SP DMA queues so the final y
                        # flush runs on two rings in parallel
                        eng = nc.scalar if (mt + nt2) % 2 else nc.sync
                        eng.dma_start(
                            y[128 * mt:128 * (mt + 1),
                              512 * nt2:512 * (nt2 + 1)],
                            yt[:])

            # ---- emission script -------------------------------------------
            # ramp-min: everything pair0-qt0 kt0-3 needs (k_enc cols 0-511,
            # q_enc cols 0-511, V0/V1), then interleave pair0-qt0's kt
            # quarters with the remaining k-enc nt slices.
            done_kq = set()

            def kq_piece(which, nt):
                # one deferred-prep piece: a folded-encoder quad-nt slice.
                # Idempotent: emitted by whichever of the queue or a unit
                # preamble reaches it first.
                if (which, nt) in done_kq:
                    return
                done_kq.add((which, nt))
                wsb, enc_dst, enc_name = {
                    "k0": (wke_sb, k_enc[0], "kenc0"),
                    "q0": (wqe_sb, q_enc[0], "qenc0"),
                    "k1": (wke_sb, k_enc[1], "kenc1"),
                    "q1": (wqe_sb, q_enc[1], "qenc1"),
                }[which]
                qd = int(which[1])
                enc_mtile_nt(wsb, enc_dst, qd, nt, enc_name)

            def need_enc(p, qt):
                # hard deps of unit (p, qt)'s kt sweep: its quad's k-enc in
                # full, plus the q-enc slice for this qt (no-ops if already
                # drained from the queue)
                qd = p // 2
                for nt in range(NT):
                    kq_piece(f"k{qd}", nt)
                kq_piece(f"q{qd}", qt)

            # ---- metronome + fill queue ------------------------------------
            # Each unit (pair, qt) emits a tight ST+exp stream (the metronome,
            # gating ACT); between kt quarters it drains fill pieces: the
            # previous unit's attV+normalize (front of queue), then deferred
            # prep (QK m-tiles/encoders, out-proj) from the back.
            from collections import deque
            fills = deque()

            def fill(n):
                for _ in range(n):
                    if fills:
                        fills.popleft()()

            def metronome(p, qt, nfill=2, direct=None):
                exs = []
                for q in range(NT):
                    for kt in range(4 * q, 4 * q + 4):
                        exs.append(st_exp_kt(p, qt, kt))
                    if direct is not None and q < NT - 1:
                        direct(q + 1)
                    fill(nfill)
                return exs

            def attv_norm_pieces(p, qt, att, exs):
                pieces = [
                    (lambda q=q: attv_kts(p, att, exs, 4 * q, 4 * q + 4))
                    for q in range(NT)
                ]
                pieces.append(lambda: normalize(p, qt, att))
                return pieces

            def out_piece(mt):
                return lambda: out_proj_mt(mt)

            # deferred prep, in first-use order, then V prefetch (fills
            # the otherwise-starved mid-kernel units; attV's inline
            # ensure_v makes any not-yet-drained piece a no-op)
            # V prefetch covers all kt: V tiles are first needed when unit
            # (0,0)'s attV drains during unit 1, so they come off the fill
            # queue instead of gating the first STs in PE program order.
            # k1/q1 encoders are only needed by unit 8 — they go last.
            for nt in range(1, NT):
                fills.append(lambda nt=nt: kq_piece("q0", nt))
            for kt in range(NT128):
                fills.append(lambda kt=kt: (ensure_v(kt), None))
            for which in ("k1", "q1"):
                for nt in range(NT):
                    fills.append(lambda w=which, nt=nt: kq_piece(w, nt))

            # minimal ramp: quad-0 K and Q over nt0; unit (0,0) interleaves
            # the k0 nt>=1 slices directly (hard dep of its kt sweep)
            kq_piece("k0", 0)
            kq_piece("q0", 0)

            order = [(0, 0), (1, 0), (0, 1), (1, 1), (0, 2), (1, 2),
                     (0, 3), (1, 3), (2, 0), (3, 0), (2, 1), (3, 1),
                     (2, 2), (3, 2), (2, 3), (3, 3)]
            for i, (p, qt) in enumerate(order):
                att = new_att(p, qt)
                if i == 0:
                    exs = metronome(p, qt, nfill=NFILL,
                                    direct=lambda nt: kq_piece("k0", nt))
                else:
                    need_enc(p, qt)
                    exs = metronome(p, qt, nfill=NFILL)
                # previous unit's attV/normalize already queued; queue ours
                # at the front so they run in the next unit's windows
                pieces = attv_norm_pieces(p, qt, att, exs)
                if i == len(order) - 1:
                    for f in pieces:
                        f()
                else:
                    fills.extendleft(reversed(pieces))
                if p == 3 and qt < NT - 1:
                    # out-proj for qt becomes legal once pair3-qt normalize
                    # is queued; drains from the back of the queue
                    for mt in range(4 * qt, 4 * qt + 4):
                        fills.append(out_piece(mt))
                if PHASE_LIMIT == "qkv" and i == 0:
                    break
            if PHASE_LIMIT == "qkv":
                continue
            # drain whatever prep/out pieces remain, then the staged tail
            while fills:
                fills.popleft()()
            out_proj_qt_staged(NT - 1)
    nc.finalize()
    return nc


_nc_cache = None


def make_in_maps(inputs):
    x = np.asarray(inputs["x"], dtype=np.float32)
    Wq = np.asarray(inputs["Wq"], dtype=np.float32)
    Wk = np.asarray(inputs["Wk"], dtype=np.float32)
    Wv = np.asarray(inputs["Wv"], dtype=np.float32)
    We = np.asarray(inputs["W_enc"], dtype=np.float32)
    Wo = np.asarray(inputs["Wo"], dtype=np.float32)

    xts = [np.ascontiguousarray(x[b].T).astype(BF) for b in range(B)]
    in_maps = []
    for c in range(NCORES):
        b, g = divmod(c, 2)
        gs = g * GD
        # fold Q/K projections into the per-head bit encoders:
        # Wqe[:, 32i:32i+32] = Wq[head i rows].T @ W_enc[head i]
        # (Q/K are only ever consumed through tanh(Qh @ W_enc[h]))
        wqe = np.empty((D, HPG * MB), np.float32)
        wke = np.empty((D, HPG * MB), np.float32)
        for i in range(HPG):
            h = g * HPG + i
            wqe[:, MB * i:MB * (i + 1)] = \
                Wq[h * HD:(h + 1) * HD, :].T @ We[h]
            wke[:, MB * i:MB * (i + 1)] = \
                Wk[h * HD:(h + 1) * HD, :].T @ We[h]
        in_maps.append({
            "xt": xts[b],
            "wqe": wqe.astype(BF),
            "wke": wke.astype(BF),
            "wv": np.ascontiguousarray(Wv[gs:gs + GD, :].T).astype(BF),
            "wo": np.ascontiguousarray(Wo[:, gs:gs + GD].T).astype(BF),
        })
    return in_maps


def kernel(**inputs):
    global _nc_cache, LAST_RESULTS
    if _nc_cache is None:
        _nc_cache = build()
    nc = _nc_cache
    in_maps = make_in_maps(inputs)

    res = run_bass_kernel_spmd(
        nc, in_maps, core_ids=list(range(NCORES)),
        trace=TRACE, **TRACE_KW)
    LAST_RESULTS = res

    out = np.empty((B, N, D), dtype=np.float32)
    for b in range(B):
        out[b] = (res.results[2 * b]["y"].astype(np.float32)
                  + res.results[2 * b + 1]["y"].astype(np.float32))
    return out



# revision 28
# speedup vs baseline: 1.2402x; 1.2402x over previous
"""BREWA (bit-witness) attention on 8 TRN2 NeuronCores.

Sharding: core c = (batch b, head-group g) with b = c // 2, g = c % 2.
Each core computes its batch's attention for 8 of the 16 heads plus the
partial output projection over those heads' Wo columns; the host sums the
two partial projections per batch (the "all-reduce" is 2-way, done on CPU).

Key structural trick: Q and K are consumed ONLY through the per-head bit
encoders tanh((x@Wq^T)_h @ W_enc[h]), so the two linear maps are folded
on the host into Wqe[h] = Wq_h^T @ W_enc[h] ([1024 -> 32] per head) —
the full-width QK projections never exist on device (-35us PE, -224
matmuls, and one less bf16 rounding).

Per-core dataflow (all matmuls bf16, fp32 PSUM accumulate):
  xT[b] (host-transposed, bf16)
    --PE (folded Wqe/Wke, K=1024)--> enc psum --ACT tanh--> q_encT,k_encT
        [128, 2048] per quad: 4 heads x 32 bits on partitions, seq free
    --PE--> V [2048,520] (seq on partitions; 65 cols/head: 64 V + ones)
  per (head-pair p, q-tile 512, k-tile 128):
    ST[k,q] via 2 row-tiled K=32 matmuls -> st psum [128, 1024]
    exp(ST/sqrt32) on ACT -> SBUF bf16   (softmax w/o max-sub: |scores|<=5.66)
    att[r] += V_aug[kt,h].T @ expST      (psum [65,512]; row 64 = sum_k exp = Z)
  normalize: DVE reciprocal(Z) -> GPSIMD partition_broadcast -> DVE mul -> c_T
  y = c_T.T @ WoT_g  (per-core partial, bf16 out; host upcasts + sums)

Scheduling (metronome + fill queue): DMA is split and ordered
(interleaved wke/xt-nt0 k-pairs, wqe, wv, xt-nt1..3, wo) so the first
encoder matmul starts ~1us in. Each unit (head-pair, qt) emits a tight
ST->exp stream — the metronome that keeps ACT saturated; between kt
quarters it drains NFILL pieces from a deque: the previous unit's
attV+normalize (queue front, one-unit lag decouples attV from exp),
then deferred encoder slices and the out-proj chase (back). Unit (0,0)
interleaves the quad-0 k-enc slices directly (hard dep of its kt
sweep). PSUM: st 2x[128,1024] + att 2x[65,512] + bank 2x[128,512] = 8
banks. The tail out-proj is staged (ready contractions ahead of the
ct-DMA-gated last k) with its y DMAs on the then-idle ACT queue.

fp8e4 DoubleRow for the scores matmul was tried and removed:
microbenchmarks measured DR at parity/slower than bf16 on real silicon
(cost model claims 2x), and it costs 3.5x the rel-err margin (archived
in kernel_v3.py).
"""

import numpy as np
import ml_dtypes

import concourse.bacc as bacc
import concourse.bass as bass
import concourse.mybir as mybir
import concourse.tile as tile
from concourse.bass_utils import run_bass_kernel_spmd

B, N, D = 4, 2048, 1024
H, HD, MB = 16, 64, 32
NCORES = 8
HPG = 8              # heads per group (per core)
GD = HPG * HD        # 512 head dims per group
SCALE = float(1.0 / np.sqrt(MB))

bf16 = mybir.dt.bfloat16
f32 = mybir.dt.float32
BF = ml_dtypes.bfloat16
AF = mybir.ActivationFunctionType

KT_X = D // 128      # 8 contraction tiles over d_model
NT = N // 512        # 4 column tiles of 512 over sequence
MT_QK = GD // 128    # 4 partition tiles of QT/KT
NT128 = N // 128     # 16 row tiles of 128 over sequence
KT_C = GD // 128     # 4 contraction tiles over group head dims

TRACE = False        # set by test.py for profiling runs
TRACE_KW = {}
LAST_RESULTS = None
PHASE_LIMIT = "full"  # "qkv" | "attn" | "full" — for sim phase ablation
NFILL = 2            # fill pieces drained per metronome quarter


def build(reps=1):
    nc = bacc.Bacc("TRN2", target_bir_lowering=False, debug=False,
                   num_devices=NCORES)
    EB = HPG * MB        # 256 encoder bits per core (8 heads x 32)
    xt = nc.dram_tensor("xt", [D, N], bf16, kind="ExternalInput").ap()
    # folded encoder weights: Wqe = Wq_h^T @ W_enc[h] per head, [1024, 256]
    # (Q/K are consumed only through the encoders, so the full-width QK
    # projections fold away entirely)
    wqe = nc.dram_tensor("wqe", [D, EB], bf16, kind="ExternalInput").ap()
    wke = nc.dram_tensor("wke", [D, EB], bf16, kind="ExternalInput").ap()
    wv = nc.dram_tensor("wv", [D, GD], bf16, kind="ExternalInput").ap()
    wo = nc.dram_tensor("wo", [GD, D], bf16, kind="ExternalInput").ap()
    y = nc.dram_tensor("y", [N, D], bf16, kind="ExternalOutput").ap()

    with tile.TileContext(nc) as tc:
        with (
            tc.tile_pool(name="xtp", bufs=KT_X) as xt_pool,
            tc.tile_pool(name="wp", bufs=3 * KT_X) as w_pool,
            tc.tile_pool(name="wop", bufs=KT_C) as wo_pool,
            tc.tile_pool(name="encp", bufs=4) as enc_pool,
            tc.tile_pool(name="vp", bufs=NT128) as v_pool,
            tc.tile_pool(name="expp", bufs=16) as exp_pool,
            tc.tile_pool(name="ctp", bufs=KT_C) as ct_pool,
            tc.tile_pool(name="smallp", bufs=4) as small_pool,
            tc.tile_pool(name="yp", bufs=6) as y_pool,
            tc.tile_pool(name="stp", bufs=2, space="PSUM") as st_pool,
            tc.tile_pool(name="attp", bufs=2, space="PSUM") as att_pool,
            tc.tile_pool(name="bankp", bufs=2, space="PSUM") as bank_pool,
        ):
          for _rep in range(reps):
            # ---- input loads: each tensor's k-slices split across BOTH
            # HWDGE queues (even k -> SP, odd k -> ACT; ACT descriptors
            # precede every ACT compute instruction in program order), in
            # consumer-priority order so both rings fill the pipe in
            # parallel at ~2x the single-queue rate ---------------------------
            def load_w(w_ap, cols, name):
                tiles = []
                for k in range(KT_X):
                    t = w_pool.tile([128, cols], bf16, tag=f"w{cols}",
                                    name=f"{name}{k}")
                    nc.sync.dma_start(t[:], w_ap[128 * k:128 * (k + 1), :])
                    tiles.append(t)
                return tiles

            xt_sb = [xt_pool.tile([128, N], bf16, tag="xt", name=f"xt{k}")
                     for k in range(KT_X)]

            def load_xt_nt(nt):
                for k in range(KT_X):
                    nc.sync.dma_start(
                        xt_sb[k][:, 512 * nt:512 * (nt + 1)],
                        xt[128 * k:128 * (k + 1), 512 * nt:512 * (nt + 1)])

            # interleave wke k-tiles with xt nt0 slices so the first
            # encoder m-tile's k-loop starts after the first pair lands
            wke_sb = []
            for k in range(KT_X):
                t = w_pool.tile([128, EB], bf16, tag="w256", name=f"wke{k}")
                nc.sync.dma_start(t[:], wke[128 * k:128 * (k + 1), :])
                wke_sb.append(t)
                nc.sync.dma_start(
                    xt_sb[k][:, 0:512],
                    xt[128 * k:128 * (k + 1), 0:512])

            wqe_sb = load_w(wqe, EB, "wqe")
            wv_sb = load_w(wv, GD, "wv")
            for nt in range(1, NT):
                load_xt_nt(nt)
            wo_sb = []
            for k in range(KT_C):
                t = wo_pool.tile([128, D], bf16, tag="wo")
                nc.sync.dma_start(t[:], wo[128 * k:128 * (k + 1), :])
                wo_sb.append(t)

            # ---- encoders: [128 bits (4 heads x 32), 2048 seq] per quad ----
            q_enc = [enc_pool.tile([128, N], bf16, tag="enc",
                                   name=f"qenc{d}") for d in range(2)]
            k_enc = [enc_pool.tile([128, N], bf16, tag="enc",
                                   name=f"kenc{d}") for d in range(2)]

            def enc_mtile_nt(wsb, et, qd, nt, name):
                # one folded x->bits projection: psum[bit, seq] -> tanh
                ps = bank_pool.tile([128, 512], f32, tag="bank",
                                    name=f"ps_{name}_{nt}")
                for k in range(KT_X):
                    nc.tensor.matmul(
                        ps[:],
                        wsb[k][:, 128 * qd:128 * (qd + 1)],
                        xt_sb[k][:, 512 * nt:512 * (nt + 1)],
                        start=(k == 0), stop=(k == KT_X - 1),
                        skip_group_check=True,
                    )
                nc.scalar.activation(et[:, 512 * nt:512 * (nt + 1)],
                                     ps[:], AF.Tanh)

            v_sb = [None] * NT128

            def ensure_v(nt):
                if v_sb[nt] is not None:
                    return v_sb[nt]
                t = v_pool.tile([128, HPG * 65], bf16, tag="v", name=f"v{nt}")
                ps = bank_pool.tile([128, 512], f32, tag="bank",
                                    name=f"ps_v{nt}")
                for k in range(KT_X):
                    nc.tensor.matmul(
                        ps[:],
                        xt_sb[k][:, 128 * nt:128 * (nt + 1)],
                        wv_sb[k][:],
                        start=(k == 0), stop=(k == KT_X - 1),
                        skip_group_check=True,
                    )
                vv = t[:, :].rearrange("p (h s) -> p h s", h=HPG)
                nc.vector.tensor_copy(
                    vv[:, :, 0:64],
                    ps[:, :].rearrange("p (h s) -> p h s", h=HPG),
                )
                nc.vector.memset(vv[:, :, 64:65], 1.0)
                v_sb[nt] = t
                return t

            # ---- c_T accumulator tiles: [512 head dims, 2048 seq] -----------
            ct_sb = [ct_pool.tile([128, N], bf16, tag="ct", name=f"ct{i}")
                     for i in range(KT_C)]

            def st_exp_kt(p, qt, kt):
                """ST -> exp for heads (2p, 2p+1) at one kt; returns ex."""
                qd = p // 2
                st = st_pool.tile([128, N // 2], f32, tag="st")
                for r in range(2):
                    a = 2 * (p % 2) + r
                    nc.tensor.matmul(
                        st[:, 512 * r:512 * (r + 1)],
                        k_enc[qd][32 * a:32 * (a + 1),
                                  128 * kt:128 * (kt + 1)],
                        q_enc[qd][32 * a:32 * (a + 1),
                                  512 * qt:512 * (qt + 1)],
                        start=True, stop=True,
                        tile_position=(32 * a, 0),
                        skip_group_check=True,
                    )
                ex = exp_pool.tile([128, N // 2], bf16, tag="exp")
                nc.scalar.activation(ex[:], st[:], AF.Exp, scale=SCALE)
                return ex

            def attv_kts(p, att, exs, kt_lo, kt_hi):
                # r=1 first: normalize frees att[1] first (its chain is
                # emitted first), so the first accumulate here waits on the
                # earlier-freed tile
                for kt in range(kt_lo, kt_hi):
                    for r in (1, 0):
                        h = 2 * p + r
                        nc.tensor.matmul(
                            att[r][0:65, :],
                            ensure_v(kt)[:, 65 * h:65 * h + 65],
                            exs[kt][:, 512 * r:512 * (r + 1)],
                            start=(kt == 0), stop=(kt == NT128 - 1),
                            skip_group_check=True,
                        )

            def normalize(p, qt, att):
                # pipelined across DVE/Pool: both recips issue first, then
                # both broadcasts, then both muls; the r=1 chain leads and
                # everything is lane-aligned (r=1 data at partitions 64-127)
                # so ct is written directly — no partition-shift DMA
                # NOTE: reciprocal_approx_fast passes CoreSim but returns
                # garbage on hw through this compile path (custom-DVE
                # table likely not shipped by the bass2jax/axon NEFF
                # build) — keep the exact reciprocal
                qs = slice(512 * qt, 512 * (qt + 1))
                rec1 = small_pool.tile([1, 512], f32, tag="recip")
                nc.vector.reciprocal(rec1[:], att[1][64:65, :])
                rec0 = small_pool.tile([1, 512], f32, tag="recip")
                nc.vector.reciprocal(rec0[:], att[0][64:65, :])
                bc1 = small_pool.tile([64, 512], f32, tag="bc")
                nc.gpsimd.partition_broadcast(bc1[:], rec1[:])
                bc0 = small_pool.tile([64, 512], f32, tag="bc")
                nc.gpsimd.partition_broadcast(bc0[:], rec0[:])
                # odd head: DVE writes the partition-shifted destination
                # directly (in base 0 -> out base 64)
                nc.vector.tensor_mul(
                    ct_sb[p][64:128, qs], att[1][0:64, :], bc1[:])
                nc.vector.tensor_mul(
                    ct_sb[p][0:64, qs], att[0][0:64, :], bc0[:])

            def new_att(p, qt):
                # full-bank tiles: r=1 uses partitions 63-127, r=0 uses 0-64
                return [att_pool.tile([128, 512], f32, tag="att",
                                      name=f"att{p}_{qt}_{r}")
                        for r in range(2)]


            def out_proj_mt(mt):
                # y rows 128*mt .. 128*(mt+1): 2 out-dim halves
                for nt2 in range(2):
                    ps = bank_pool.tile([128, 512], f32, tag="bank",
                                        name=f"ps_y{mt}_{nt2}")
                    for k in range(KT_C):
                        nc.tensor.matmul(
                            ps[:],
                            ct_sb[k][:, 128 * mt:128 * (mt + 1)],
                            wo_sb[k][:, 512 * nt2:512 * (nt2 + 1)],
                            start=(k == 0), stop=(k == KT_C - 1),
                            skip_group_check=True,
                        )
                    yt = y_pool.tile([128, 512], bf16, tag="y")
                    nc.vector.tensor_copy(yt[:], ps[:])
                    nc.sync.dma_start(
                        y[128 * mt:128 * (mt + 1),
                          512 * nt2:512 * (nt2 + 1)],
                        yt[:])

            def out_proj_qt(qt):
                for mt in range(4 * qt, 4 * qt + 4):
                    out_proj_mt(mt)

            def out_proj_qt_staged(qt):
                # tail variant: PE executes in order, so emit k=0..2 of two
                # groups before their k=3 (which waits on the last ct DMA);
                # the ready contractions fill the wait.
                slots = [(mt, nt2) for mt in range(4 * qt, 4 * qt + 4)
                         for nt2 in range(2)]
                for i in range(0, len(slots), 2):
                    pss = []
                    for mt, nt2 in slots[i:i + 2]:
                        ps = bank_pool.tile([128, 512], f32, tag="bank",
                                            name=f"ps_y{mt}_{nt2}")
                        for k in range(KT_C - 1):
                            nc.tensor.matmul(
                                ps[:],
                                ct_sb[k][:, 128 * mt:128 * (mt + 1)],
                                wo_sb[k][:, 512 * nt2:512 * (nt2 + 1)],
                                start=(k == 0), stop=False,
                                skip_group_check=True,
                            )
                        pss.append(ps)
                    for (mt, nt2), ps in zip(slots[i:i + 2], pss):
                        nc.tensor.matmul(
                            ps[:],
                            ct_sb[KT_C - 1][:, 128 * mt:128 * (mt + 1)],
                            wo_sb[KT_C - 1][:, 512 * nt2:512 * (nt2 + 1)],
                            start=False, stop=True,
                            skip_group_check=True,
                        )
                        yt = y_pool.tile([128, 512], bf16, tag="y")
                        # tail flush: alternate the psum->sbuf copies between
                        # DVE and ACT (both idle here) and the y DMAs between
                        # the SP/ACT rings, so neither engine serializes the
                        # last 8 tiles
                        if (mt + nt2) % 2:
                            nc.scalar.activation(yt[:], ps[:], AF.Copy)
                            nc.scalar.dma_start(
                                y[128 * mt:128 * (mt + 1),
                                  512 * nt2:512 * (nt2 + 1)],
                                yt[:])
                        else:
                            nc.vector.tensor_copy(yt[:], ps[:])
                            nc.sync.dma_start(
                                y[128 * mt:128 * (mt + 1),
                                  512 * nt2:512 * (nt2 + 1)],
                                yt[:])

            # ---- emission script -------------------------------------------
            # ramp-min: everything pair0-qt0 kt0-3 needs (k_enc cols 0-511,
            # q_enc cols 0-511, V0/V1), then interleave pair0-qt0's kt
            # quarters with the remaining k-enc nt slices.
            done_kq = set()

            def kq_piece(which, nt):
                # one deferred-prep piece: a folded-encoder quad-nt slice.
                # Idempotent: emitted by whichever of the queue or a unit
                # preamble reaches it first.
                if (which, nt) in done_kq:
                    return
                done_kq.add((which, nt))
                wsb, enc_dst, enc_name = {
                    "k0": (wke_sb, k_enc[0], "kenc0"),
                    "q0": (wqe_sb, q_enc[0], "qenc0"),
                    "k1": (wke_sb, k_enc[1], "kenc1"),
                    "q1": (wqe_sb, q_enc[1], "qenc1"),
                }[which]
                qd = int(which[1])
                enc_mtile_nt(wsb, enc_dst, qd, nt, enc_name)

            def need_enc(p, qt):
                # hard deps of unit (p, qt)'s kt sweep: its quad's k-enc in
                # full, plus the q-enc slice for this qt (no-ops if already
                # drained from the queue)
                qd = p // 2
                for nt in range(NT):
                    kq_piece(f"k{qd}", nt)
                kq_piece(f"q{qd}", qt)

            # ---- metronome + fill queue ------------------------------------
            # Each unit (pair, qt) emits a tight ST+exp stream (the metronome,
            # gating ACT); between kt quarters it drains fill pieces: the
            # previous unit's attV+normalize (front of queue), then deferred
            # prep (QK m-tiles/encoders, out-proj) from the back.
            from collections import deque
            fills = deque()

            def fill(n):
                for _ in range(n):
                    if fills:
                        fills.popleft()()

            def metronome(p, qt, nfill=2, direct=None):
                exs = []
                for q in range(NT):
                    for kt in range(4 * q, 4 * q + 4):
                        exs.append(st_exp_kt(p, qt, kt))
                    if direct is not None and q < NT - 1:
                        direct(q + 1)
                    fill(nfill)
                return exs

            def attv_norm_pieces(p, qt, att, exs):
                pieces = [
                    (lambda q=q: attv_kts(p, att, exs, 4 * q, 4 * q + 4))
                    for q in range(NT)
                ]
                pieces.append(lambda: normalize(p, qt, att))
                return pieces

            def out_piece(mt):
                return lambda: out_proj_mt(mt)

            # deferred prep, in first-use order, then V prefetch (fills
            # the otherwise-starved mid-kernel units; attV's inline
            # ensure_v makes any not-yet-drained piece a no-op)
            # only the pieces needed soon go in up front; the rest are
            # staggered into the back half of the unit loop (below), where
            # the fill queue otherwise runs dry and the metronome's 1-kt
            # exp lookahead stalls PE ~186ns per kt
            fills.append(lambda: kq_piece("q0", 1))
            for kt in range(2, NT128):
                fills.append(lambda kt=kt: (ensure_v(kt), None))
            deferred = {
                1: [lambda: kq_piece("q0", 2)],
                3: [lambda: kq_piece("q0", 3)] + [
                    lambda nt=nt: kq_piece("k1", nt) for nt in range(NT)],
                5: [lambda: kq_piece("q1", 0)],
                7: [lambda: kq_piece("q1", 1)],
                9: [lambda: kq_piece("q1", 2)],
                11: [lambda: kq_piece("q1", 3)],
            }

            # minimal ramp: quad-0 K and Q over nt0, V0/V1; unit (0,0)
            # interleaves the k0 nt>=1 slices directly (hard dep of its kt
            # sweep)
            kq_piece("k0", 0)
            kq_piece("q0", 0)
            ensure_v(0)
            ensure_v(1)

            order = [(0, 0), (1, 0), (0, 1), (1, 1), (0, 2), (1, 2),
                     (0, 3), (1, 3), (2, 0), (3, 0), (2, 1), (3, 1),
                     (2, 2), (3, 2), (2, 3), (3, 3)]
            for i, (p, qt) in enumerate(order):
                att = new_att(p, qt)
                if i == 0:
                    exs = metronome(p, qt, nfill=NFILL,
                                    direct=lambda nt: kq_piece("k0", nt))
                else:
                    need_enc(p, qt)
                    exs = metronome(p, qt, nfill=NFILL)
                # previous unit's attV/normalize already queued; queue ours
                # at the front so they run in the next unit's windows
                pieces = attv_norm_pieces(p, qt, att, exs)
                if i == len(order) - 1:
                    # tail: attV inline, then the remaining fills (their ct
                    # reads must precede normalize(15)'s ct writes — tile-
                    # granular deps would otherwise serialize them after the
                    # whole chain), then normalize, then the staged out-proj
                    for f in pieces[:-1]:
                        f()
                    while fills:
                        fills.popleft()()
                    pieces[-1]()
                else:
                    fills.extendleft(reversed(pieces))
                if p == 3 and qt < NT - 1:
                    # out-proj for qt becomes legal once pair3-qt normalize
                    # is queued; drains from the back of the queue
                    for mt in range(4 * qt, 4 * qt + 4):
                        fills.append(out_piece(mt))
                for f in deferred.get(i, ()):
                    fills.append(f)
                if PHASE_LIMIT == "qkv" and i == 0:
                    break
            if PHASE_LIMIT == "qkv":
                continue
            out_proj_qt_staged(NT - 1)
    nc.finalize()
    return nc


_nc_cache = None


def make_in_maps(inputs):
    x = np.asarray(inputs["x"], dtype=np.float32)
    Wq = np.asarray(inputs["Wq"], dtype=np.float32)
    Wk = np.asarray(inputs["Wk"], dtype=np.float32)
    Wv = np.asarray(inputs["Wv"], dtype=np.float32)
    We = np.asarray(inputs["W_enc"], dtype=np.float32)
    Wo = np.asarray(inputs["Wo"], dtype=np.float32)

    xts = [np.ascontiguousarray(x[b].T).astype(BF) for b in range(B)]
    in_maps = []
    for c in range(NCORES):
        b, g = divmod(c, 2)
        gs = g * GD
        # fold Q/K projections into the per-head bit encoders:
        # Wqe[:, 32i:32i+32] = Wq[head i rows].T @ W_enc[head i]
        # (Q/K are only ever consumed through tanh(Qh @ W_enc[h]))
        wqe = np.empty((D, HPG * MB), np.float32)
        wke = np.empty((D, HPG * MB), np.float32)
        for i in range(HPG):
            h = g * HPG + i
            wqe[:, MB * i:MB * (i + 1)] = \
                Wq[h * HD:(h + 1) * HD, :].T @ We[h]
            wke[:, MB * i:MB * (i + 1)] = \
                Wk[h * HD:(h + 1) * HD, :].T @ We[h]
        in_maps.append({
            "xt": xts[b],
            "wqe": wqe.astype(BF),
            "wke": wke.astype(BF),
            "wv": np.ascontiguousarray(Wv[gs:gs + GD, :].T).astype(BF),
            "wo": np.ascontiguousarray(Wo[:, gs:gs + GD].T).astype(BF),
        })
    return in_maps


def kernel(**inputs):
    global _nc_cache, LAST_RESULTS
    if _nc_cache is None:
        _nc_cache = build()
    nc = _nc_cache
    in_maps = make_in_maps(inputs)

    res = run_bass_kernel_spmd(
        nc, in_maps, core_ids=list(range(NCORES)),
        trace=TRACE, **TRACE_KW)
    LAST_RESULTS = res

    out = np.empty((B, N, D), dtype=np.float32)
    for b in range(B):
        out[b] = (res.results[2 * b]["y"].astype(np.float32)
                  + res.results[2 * b + 1]["y"].astype(np.float32))
    return out



# revision 29
# speedup vs baseline: 11.8377x; 9.5451x over previous
"""BREWA (bit-witness) attention on 8 TRN2 NeuronCores.

Sharding: core c = (batch b, head-group g) with b = c // 2, g = c % 2.
Each core computes its batch's attention for 8 of the 16 heads plus the
partial output projection over those heads' Wo columns; the host sums the
two partial projections per batch (the "all-reduce" is 2-way, done on CPU).

Key structural trick: Q and K are consumed ONLY through the per-head bit
encoders tanh((x@Wq^T)_h @ W_enc[h]), so the two linear maps are folded
on the host into Wqe[h] = Wq_h^T @ W_enc[h] ([1024 -> 32] per head) —
the full-width QK projections never exist on device (-35us PE, -224
matmuls, and one less bf16 rounding).

Per-core dataflow (all matmuls bf16, fp32 PSUM accumulate):
  xT[b] (host-transposed, bf16)
    --PE (folded Wqe/Wke, K=1024)--> enc psum --ACT tanh--> q_encT,k_encT
        [128, 2048] per quad: 4 heads x 32 bits on partitions, seq free
    --PE--> V [2048,520] (seq on partitions; 65 cols/head: 64 V + ones)
  per (head-pair p, q-tile 512, k-tile 128):
    ST[k,q] via 2 row-tiled K=32 matmuls -> st psum [128, 1024]
    exp(ST/sqrt32) on ACT -> SBUF bf16   (softmax w/o max-sub: |scores|<=5.66)
    att[r] += V_aug[kt,h].T @ expST      (psum [65,512]; row 64 = sum_k exp = Z)
  normalize (pipelined across DVE/Pool; r=1 chain leads since attV
    accumulates r=1 first): 2x DVE reciprocal(Z) -> 2x GPSIMD
    partition_broadcast -> 2x DVE mul -> c_T. The odd head's mul writes
    ct rows 64-127 DIRECTLY (DVE accepts differing in/out partition
    bases, verified on hw) — the old SBUF partition-shift DMA and its
    ~2.7us queue+completion latency are gone.
  y = c_T.T @ WoT_g  (per-core partial, bf16 out; host upcasts + sums)

Scheduling (metronome + fill queue): DMA is split and ordered
(interleaved wke/xt-nt0 k-pairs, wqe, wv, xt-nt1..3, wo) so the first
encoder matmul starts ~1us in. Each unit (head-pair, qt) emits a tight
ST->exp stream — the metronome that keeps ACT saturated; between kt
quarters it drains NFILL pieces from a deque: the previous unit's
attV+normalize (queue front, one-unit lag decouples attV from exp),
then deferred prep from the back. Deferred encoder pieces (q0-nt2/3,
k1, q1-nt*) are staggered into the unit loop just before first use —
the back half (units 8-14) otherwise runs dry of fill work and the
1-kt exp lookahead stalls PE ~186ns per kt. Unit (0,0) interleaves the
quad-0 k-enc slices directly (hard dep of its kt sweep). PSUM: st
2x[128,1024] + att 2x[65,512] + bank 2x[128,512] = 8 banks. Tail: attV
of the last unit runs inline, remaining fills drain BEFORE its
normalize (tile-granular ct deps would otherwise serialize them after
the whole chain), then the staged out-proj (ready k<3 contractions
ahead of the normalize-gated last k) with psum->sbuf copies alternated
DVE/ACT and y DMAs alternated across the SP/ACT rings.

Probe-measured hw behavior (timing-only builds): halving attV matmuls
transfers ~1:1 to total time, halving ST only ~40%, halving exp ~22% —
per-kt PE work (ST+attV+fills ~1.45us) is the binding rate with ACT
(~1.14us/kt) close behind; both engines carry ~+40ns/instr of
LDW/sem overhead vs the cost model.

fp8e4 DoubleRow for the scores matmul was tried and removed:
microbenchmarks measured DR at parity/slower than bf16 on real silicon
(cost model claims 2x), and it costs 3.5x the rel-err margin. walrus
--enable-ldw-opt=true fails in visitInstLdweights (that's why the
compile path pins it false).
"""

import numpy as np
import ml_dtypes

import concourse.bacc as bacc
import concourse.bass as bass
import concourse.mybir as mybir
import concourse.tile as tile
from concourse.bass_utils import run_bass_kernel_spmd

B, N, D = 4, 2048, 1024
H, HD, MB = 16, 64, 32
NCORES = 8
HPG = 8              # heads per group (per core)
GD = HPG * HD        # 512 head dims per group
SCALE = float(1.0 / np.sqrt(MB))

bf16 = mybir.dt.bfloat16
f32 = mybir.dt.float32
BF = ml_dtypes.bfloat16
AF = mybir.ActivationFunctionType

KT_X = D // 128      # 8 contraction tiles over d_model
NT = N // 512        # 4 column tiles of 512 over sequence
MT_QK = GD // 128    # 4 partition tiles of QT/KT
NT128 = N // 128     # 16 row tiles of 128 over sequence
KT_C = GD // 128     # 4 contraction tiles over group head dims

TRACE = False        # set by test.py for profiling runs
TRACE_KW = {}
LAST_RESULTS = None
PHASE_LIMIT = "full"  # "qkv" | "attn" | "full" — for sim phase ablation
NFILL = 2            # fill pieces drained per metronome quarter


def build(reps=1):
    nc = bacc.Bacc("TRN2", target_bir_lowering=False, debug=False,
                   num_devices=NCORES)
    EB = HPG * MB        # 256 encoder bits per core (8 heads x 32)
    xt = nc.dram_tensor("xt", [D, N], bf16, kind="ExternalInput").ap()
    # folded encoder weights: Wqe = Wq_h^T @ W_enc[h] per head, [1024, 256]
    # (Q/K are consumed only through the encoders, so the full-width QK
    # projections fold away entirely)
    wqe = nc.dram_tensor("wqe", [D, EB], bf16, kind="ExternalInput").ap()
    wke = nc.dram_tensor("wke", [D, EB], bf16, kind="ExternalInput").ap()
    wv = nc.dram_tensor("wv", [D, GD], bf16, kind="ExternalInput").ap()
    wo = nc.dram_tensor("wo", [GD, D], bf16, kind="ExternalInput").ap()
    y = nc.dram_tensor("y", [N, D], bf16, kind="ExternalOutput").ap()

    with tile.TileContext(nc) as tc:
        with (
            tc.tile_pool(name="xtp", bufs=KT_X) as xt_pool,
            tc.tile_pool(name="wp", bufs=3 * KT_X) as w_pool,
            tc.tile_pool(name="wop", bufs=KT_C) as wo_pool,
            tc.tile_pool(name="encp", bufs=4) as enc_pool,
            tc.tile_pool(name="vp", bufs=NT128) as v_pool,
            tc.tile_pool(name="expp", bufs=16) as exp_pool,
            tc.tile_pool(name="ctp", bufs=KT_C) as ct_pool,
            tc.tile_pool(name="smallp", bufs=4) as small_pool,
            tc.tile_pool(name="yp", bufs=6) as y_pool,
            tc.tile_pool(name="stp", bufs=2, space="PSUM") as st_pool,
            tc.tile_pool(name="attp", bufs=2, space="PSUM") as att_pool,
            tc.tile_pool(name="bankp", bufs=2, space="PSUM") as bank_pool,
        ):
          for _rep in range(reps):
            # ---- input loads: each tensor's k-slices split across BOTH
            # HWDGE queues (even k -> SP, odd k -> ACT; ACT descriptors
            # precede every ACT compute instruction in program order), in
            # consumer-priority order so both rings fill the pipe in
            # parallel at ~2x the single-queue rate ---------------------------
            def load_w(w_ap, cols, name):
                tiles = []
                for k in range(KT_X):
                    t = w_pool.tile([128, cols], bf16, tag=f"w{cols}",
                                    name=f"{name}{k}")
                    nc.sync.dma_start(t[:], w_ap[128 * k:128 * (k + 1), :])
                    tiles.append(t)
                return tiles

            xt_sb = [xt_pool.tile([128, N], bf16, tag="xt", name=f"xt{k}")
                     for k in range(KT_X)]

            def load_xt_nt(nt):
                for k in range(KT_X):
                    nc.sync.dma_start(
                        xt_sb[k][:, 512 * nt:512 * (nt + 1)],
                        xt[128 * k:128 * (k + 1), 512 * nt:512 * (nt + 1)])

            # interleave wke k-tiles with xt nt0 slices so the first
            # encoder m-tile's k-loop starts after the first pair lands
            wke_sb = []
            for k in range(KT_X):
                t = w_pool.tile([128, EB], bf16, tag="w256", name=f"wke{k}")
                nc.sync.dma_start(t[:], wke[128 * k:128 * (k + 1), :])
                wke_sb.append(t)
                nc.sync.dma_start(
                    xt_sb[k][:, 0:512],
                    xt[128 * k:128 * (k + 1), 0:512])

            wqe_sb = load_w(wqe, EB, "wqe")
            wv_sb = load_w(wv, GD, "wv")
            for nt in range(1, NT):
                load_xt_nt(nt)
            wo_sb = []
            for k in range(KT_C):
                t = wo_pool.tile([128, D], bf16, tag="wo")
                nc.sync.dma_start(t[:], wo[128 * k:128 * (k + 1), :])
                wo_sb.append(t)

            # ---- encoders: [128 bits (4 heads x 32), 2048 seq] per quad ----
            q_enc = [enc_pool.tile([128, N], bf16, tag="enc",
                                   name=f"qenc{d}") for d in range(2)]
            k_enc = [enc_pool.tile([128, N], bf16, tag="enc",
                                   name=f"kenc{d}") for d in range(2)]

            def enc_mtile_nt(wsb, et, qd, nt, name):
                # one folded x->bits projection: psum[bit, seq] -> tanh
                ps = bank_pool.tile([128, 512], f32, tag="bank",
                                    name=f"ps_{name}_{nt}")
                for k in range(KT_X):
                    nc.tensor.matmul(
                        ps[:],
                        wsb[k][:, 128 * qd:128 * (qd + 1)],
                        xt_sb[k][:, 512 * nt:512 * (nt + 1)],
                        start=(k == 0), stop=(k == KT_X - 1),
                        skip_group_check=True,
                    )
                nc.scalar.activation(et[:, 512 * nt:512 * (nt + 1)],
                                     ps[:], AF.Tanh)

            v_sb = [None] * NT128

            def ensure_v(nt):
                if v_sb[nt] is not None:
                    return v_sb[nt]
                t = v_pool.tile([128, HPG * 65], bf16, tag="v", name=f"v{nt}")
                ps = bank_pool.tile([128, 512], f32, tag="bank",
                                    name=f"ps_v{nt}")
                for k in range(KT_X):
                    nc.tensor.matmul(
                        ps[:],
                        xt_sb[k][:, 128 * nt:128 * (nt + 1)],
                        wv_sb[k][:],
                        start=(k == 0), stop=(k == KT_X - 1),
                        skip_group_check=True,
                    )
                vv = t[:, :].rearrange("p (h s) -> p h s", h=HPG)
                nc.vector.tensor_copy(
                    vv[:, :, 0:64],
                    ps[:, :].rearrange("p (h s) -> p h s", h=HPG),
                )
                nc.vector.memset(vv[:, :, 64:65], 1.0)
                v_sb[nt] = t
                return t

            # ---- c_T accumulator tiles: [512 head dims, 2048 seq] -----------
            ct_sb = [ct_pool.tile([128, N], bf16, tag="ct", name=f"ct{i}")
                     for i in range(KT_C)]

            def st_exp_kt(p, qt, kt):
                """ST -> exp for heads (2p, 2p+1) at one kt; returns ex."""
                qd = p // 2
                st = st_pool.tile([128, N // 2], f32, tag="st")
                for r in range(2):
                    a = 2 * (p % 2) + r
                    nc.tensor.matmul(
                        st[:, 512 * r:512 * (r + 1)],
                        k_enc[qd][32 * a:32 * (a + 1),
                                  128 * kt:128 * (kt + 1)],
                        q_enc[qd][32 * a:32 * (a + 1),
                                  512 * qt:512 * (qt + 1)],
                        start=True, stop=True,
                        tile_position=(32 * a, 0),
                        skip_group_check=True,
                    )
                ex = exp_pool.tile([128, N // 2], bf16, tag="exp")
                nc.scalar.activation(ex[:], st[:], AF.Exp, scale=SCALE)
                return ex

            def attv_kts(p, att, exs, kt_lo, kt_hi):
                # r=1 first: normalize frees att[1] first (its chain is
                # emitted first), so the first accumulate here waits on the
                # earlier-freed tile
                for kt in range(kt_lo, kt_hi):
                    for r in (1, 0):
                        h = 2 * p + r
                        nc.tensor.matmul(
                            att[r][0:65, :],
                            ensure_v(kt)[:, 65 * h:65 * h + 65],
                            exs[kt][:, 512 * r:512 * (r + 1)],
                            start=(kt == 0), stop=(kt == NT128 - 1),
                            skip_group_check=True,
                        )

            def normalize(p, qt, att):
                # pipelined across DVE/Pool: both recips issue first, then
                # both broadcasts, then both muls; the r=1 chain leads and
                # everything is lane-aligned (r=1 data at partitions 64-127)
                # so ct is written directly — no partition-shift DMA
                # NOTE: reciprocal_approx_fast passes CoreSim but returns
                # garbage on hw through this compile path (custom-DVE
                # table likely not shipped by the bass2jax/axon NEFF
                # build) — keep the exact reciprocal
                qs = slice(512 * qt, 512 * (qt + 1))
                rec1 = small_pool.tile([1, 512], f32, tag="recip")
                nc.vector.reciprocal(rec1[:], att[1][64:65, :])
                rec0 = small_pool.tile([1, 512], f32, tag="recip")
                nc.vector.reciprocal(rec0[:], att[0][64:65, :])
                bc1 = small_pool.tile([64, 512], f32, tag="bc")
                nc.gpsimd.partition_broadcast(bc1[:], rec1[:])
                bc0 = small_pool.tile([64, 512], f32, tag="bc")
                nc.gpsimd.partition_broadcast(bc0[:], rec0[:])
                # odd head: DVE writes the partition-shifted destination
                # directly (in base 0 -> out base 64)
                nc.vector.tensor_mul(
                    ct_sb[p][64:128, qs], att[1][0:64, :], bc1[:])
                nc.vector.tensor_mul(
                    ct_sb[p][0:64, qs], att[0][0:64, :], bc0[:])

            def new_att(p, qt):
                # full-bank tiles: r=1 uses partitions 63-127, r=0 uses 0-64
                return [att_pool.tile([128, 512], f32, tag="att",
                                      name=f"att{p}_{qt}_{r}")
                        for r in range(2)]


            def out_proj_mt(mt):
                # y rows 128*mt .. 128*(mt+1): 2 out-dim halves
                for nt2 in range(2):
                    ps = bank_pool.tile([128, 512], f32, tag="bank",
                                        name=f"ps_y{mt}_{nt2}")
                    for k in range(KT_C):
                        nc.tensor.matmul(
                            ps[:],
                            ct_sb[k][:, 128 * mt:128 * (mt + 1)],
                            wo_sb[k][:, 512 * nt2:512 * (nt2 + 1)],
                            start=(k == 0), stop=(k == KT_C - 1),
                            skip_group_check=True,
                        )
                    yt = y_pool.tile([128, 512], bf16, tag="y")
                    nc.vector.tensor_copy(yt[:], ps[:])
                    nc.sync.dma_start(
                        y[128 * mt:128 * (mt + 1),
                          512 * nt2:512 * (nt2 + 1)],
                        yt[:])

            def out_proj_qt(qt):
                for mt in range(4 * qt, 4 * qt + 4):
                    out_proj_mt(mt)

            def out_proj_qt_staged(qt):
                # tail variant: PE executes in order, so emit k=0..2 of two
                # groups before their k=3 (which waits on the last ct DMA);
                # the ready contractions fill the wait.
                slots = [(mt, nt2) for mt in range(4 * qt, 4 * qt + 4)
                         for nt2 in range(2)]
                for i in range(0, len(slots), 2):
                    pss = []
                    for mt, nt2 in slots[i:i + 2]:
                        ps = bank_pool.tile([128, 512], f32, tag="bank",
                                            name=f"ps_y{mt}_{nt2}")
                        for k in range(KT_C - 1):
                            nc.tensor.matmul(
                                ps[:],
                                ct_sb[k][:, 128 * mt:128 * (mt + 1)],
                                wo_sb[k][:, 512 * nt2:512 * (nt2 + 1)],
                                start=(k == 0), stop=False,
                                skip_group_check=True,
                            )
                        pss.append(ps)
                    for (mt, nt2), ps in zip(slots[i:i + 2], pss):
                        nc.tensor.matmul(
                            ps[:],
                            ct_sb[KT_C - 1][:, 128 * mt:128 * (mt + 1)],
                            wo_sb[KT_C - 1][:, 512 * nt2:512 * (nt2 + 1)],
                            start=False, stop=True,
                            skip_group_check=True,
                        )
                        yt = y_pool.tile([128, 512], bf16, tag="y")
                        # tail flush: alternate the psum->sbuf copies between
                        # DVE and ACT (both idle here) and the y DMAs between
                        # the SP/ACT rings, so neither engine serializes the
                        # last 8 tiles
                        if (mt + nt2) % 2:
                            nc.scalar.activation(yt[:], ps[:], AF.Copy)
                            nc.scalar.dma_start(
                                y[128 * mt:128 * (mt + 1),
                                  512 * nt2:512 * (nt2 + 1)],
                                yt[:])
                        else:
                            nc.vector.tensor_copy(yt[:], ps[:])
                            nc.sync.dma_start(
                                y[128 * mt:128 * (mt + 1),
                                  512 * nt2:512 * (nt2 + 1)],
                                yt[:])

            # ---- emission script -------------------------------------------
            # ramp-min: everything pair0-qt0 kt0-3 needs (k_enc cols 0-511,
            # q_enc cols 0-511, V0/V1), then interleave pair0-qt0's kt
            # quarters with the remaining k-enc nt slices.
            done_kq = set()

            def kq_piece(which, nt):
                # one deferred-prep piece: a folded-encoder quad-nt slice.
                # Idempotent: emitted by whichever of the queue or a unit
                # preamble reaches it first.
                if (which, nt) in done_kq:
                    return
                done_kq.add((which, nt))
                wsb, enc_dst, enc_name = {
                    "k0": (wke_sb, k_enc[0], "kenc0"),
                    "q0": (wqe_sb, q_enc[0], "qenc0"),
                    "k1": (wke_sb, k_enc[1], "kenc1"),
                    "q1": (wqe_sb, q_enc[1], "qenc1"),
                }[which]
                qd = int(which[1])
                enc_mtile_nt(wsb, enc_dst, qd, nt, enc_name)

            def need_enc(p, qt):
                # hard deps of unit (p, qt)'s kt sweep: its quad's k-enc in
                # full, plus the q-enc slice for this qt (no-ops if already
                # drained from the queue)
                qd = p // 2
                for nt in range(NT):
                    kq_piece(f"k{qd}", nt)
                kq_piece(f"q{qd}", qt)

            # ---- metronome + fill queue ------------------------------------
            # Each unit (pair, qt) emits a tight ST+exp stream (the metronome,
            # gating ACT); between kt quarters it drains fill pieces: the
            # previous unit's attV+normalize (front of queue), then deferred
            # prep (QK m-tiles/encoders, out-proj) from the back.
            from collections import deque
            fills = deque()

            def fill(n):
                for _ in range(n):
                    if fills:
                        fills.popleft()()

            def metronome(p, qt, nfill=2, direct=None):
                exs = []
                for q in range(NT):
                    for kt in range(4 * q, 4 * q + 4):
                        exs.append(st_exp_kt(p, qt, kt))
                    if direct is not None and q < NT - 1:
                        direct(q + 1)
                    fill(nfill)
                return exs

            def attv_norm_pieces(p, qt, att, exs):
                pieces = [
                    (lambda q=q: attv_kts(p, att, exs, 4 * q, 4 * q + 4))
                    for q in range(NT)
                ]
                pieces.append(lambda: normalize(p, qt, att))
                return pieces

            def out_piece(mt):
                return lambda: out_proj_mt(mt)

            # deferred prep, in first-use order, then V prefetch (fills
            # the otherwise-starved mid-kernel units; attV's inline
            # ensure_v makes any not-yet-drained piece a no-op)
            # only the pieces needed soon go in up front; the rest are
            # staggered into the back half of the unit loop (below), where
            # the fill queue otherwise runs dry and the metronome's 1-kt
            # exp lookahead stalls PE ~186ns per kt
            fills.append(lambda: kq_piece("q0", 1))
            for kt in range(2, NT128):
                fills.append(lambda kt=kt: (ensure_v(kt), None))
            deferred = {
                1: [lambda: kq_piece("q0", 2)],
                3: [lambda: kq_piece("q0", 3)] + [
                    lambda nt=nt: kq_piece("k1", nt) for nt in range(NT)],
                5: [lambda: kq_piece("q1", 0)],
                7: [lambda: kq_piece("q1", 1)],
                9: [lambda: kq_piece("q1", 2)],
                11: [lambda: kq_piece("q1", 3)],
            }

            # minimal ramp: quad-0 K and Q over nt0, V0/V1; unit (0,0)
            # interleaves the k0 nt>=1 slices directly (hard dep of its kt
            # sweep)
            kq_piece("k0", 0)
            kq_piece("q0", 0)
            ensure_v(0)
            ensure_v(1)

            order = [(0, 0), (1, 0), (0, 1), (1, 1), (0, 2), (1, 2),
                     (0, 3), (1, 3), (2, 0), (3, 0), (2, 1), (3, 1),
                     (2, 2), (3, 2), (2, 3), (3, 3)]
            for i, (p, qt) in enumerate(order):
                att = new_att(p, qt)
                if i == 0:
                    exs = metronome(p, qt, nfill=NFILL,
                                    direct=lambda nt: kq_piece("k0", nt))
                else:
                    need_enc(p, qt)
                    exs = metronome(p, qt, nfill=NFILL)
                # previous unit's attV/normalize already queued; queue ours
                # at the front so they run in the next unit's windows
                pieces = attv_norm_pieces(p, qt, att, exs)
                if i == len(order) - 1:
                    # tail: attV inline, then the remaining fills (their ct
                    # reads must precede normalize(15)'s ct writes — tile-
                    # granular deps would otherwise serialize them after the
                    # whole chain), then normalize, then the staged out-proj
                    for f in pieces[:-1]:
                        f()
                    while fills:
                        fills.popleft()()
                    pieces[-1]()
                else:
                    fills.extendleft(reversed(pieces))
                if p == 3 and qt < NT - 1:
                    # out-proj for qt becomes legal once pair3-qt normalize
                    # is queued; drains from the back of the queue
                    for mt in range(4 * qt, 4 * qt + 4):
                        fills.append(out_piece(mt))
                for f in deferred.get(i, ()):
                    fills.append(f)
                if PHASE_LIMIT == "qkv" and i == 0:
                    break
            if PHASE_LIMIT == "qkv":
                continue
            out_proj_qt_staged(NT - 1)
    nc.finalize()
    return nc


_nc_cache = None


def make_in_maps(inputs):
    x = np.asarray(inputs["x"], dtype=np.float32)
    Wq = np.asarray(inputs["Wq"], dtype=np.float32)
    Wk = np.asarray(inputs["Wk"], dtype=np.float32)
    Wv = np.asarray(inputs["Wv"], dtype=np.float32)
    We = np.asarray(inputs["W_enc"], dtype=np.float32)
    Wo = np.asarray(inputs["Wo"], dtype=np.float32)

    xts = [np.ascontiguousarray(x[b].T).astype(BF) for b in range(B)]
    in_maps = []
    for c in range(NCORES):
        b, g = divmod(c, 2)
        gs = g * GD
        # fold Q/K projections into the per-head bit encoders:
        # Wqe[:, 32i:32i+32] = Wq[head i rows].T @ W_enc[head i]
        # (Q/K are only ever consumed through tanh(Qh @ W_enc[h]))
        wqe = np.empty((D, HPG * MB), np.float32)
        wke = np.empty((D, HPG * MB), np.float32)
        for i in range(HPG):
            h = g * HPG + i
            wqe[:, MB * i:MB * (i + 1)] = \
                Wq[h * HD:(h + 1) * HD, :].T @ We[h]
            wke[:, MB * i:MB * (i + 1)] = \
                Wk[h * HD:(h + 1) * HD, :].T @ We[h]
        in_maps.append({
            "xt": xts[b],
            "wqe": wqe.astype(BF),
            "wke": wke.astype(BF),
            "wv": np.ascontiguousarray(Wv[gs:gs + GD, :].T).astype(BF),
            "wo": np.ascontiguousarray(Wo[:, gs:gs + GD].T).astype(BF),
        })
    return in_maps


def kernel(**inputs):
    global _nc_cache, LAST_RESULTS
    if _nc_cache is None:
        _nc_cache = build()
    nc = _nc_cache
    in_maps = make_in_maps(inputs)

    res = run_bass_kernel_spmd(
        nc, in_maps, core_ids=list(range(NCORES)),
        trace=TRACE, **TRACE_KW)
    LAST_RESULTS = res

    out = np.empty((B, N, D), dtype=np.float32)
    for b in range(B):
        out[b] = (res.results[2 * b]["y"].astype(np.float32)
                  + res.results[2 * b + 1]["y"].astype(np.float32))
    return out



# revision 31
# speedup vs baseline: 14.6953x; 1.2414x over previous
"""BREWA (bit-witness) attention on 8 TRN2 NeuronCores.

Sharding: core c = (batch b, head-group g) with b = c // 2, g = c % 2.
Each core computes its batch's attention for 8 of the 16 heads plus the
partial output projection over those heads' Wo columns; the host sums the
two partial projections per batch (the "all-reduce" is 2-way, done on CPU).

Key structural trick: Q and K are consumed ONLY through the per-head bit
encoders tanh((x@Wq^T)_h @ W_enc[h]), so the two linear maps are folded
on the host into Wqe[h] = Wq_h^T @ W_enc[h] ([1024 -> 32] per head) —
the full-width QK projections never exist on device (-35us PE, -224
matmuls, and one less bf16 rounding).

Per-core dataflow (all matmuls bf16, fp32 PSUM accumulate):
  xT[b] (host-transposed, bf16)
    --PE (folded Wqe/Wke, K=1024)--> enc psum --ACT tanh--> q_encT,k_encT
        [128, 2048] per quad: 4 heads x 32 bits on partitions, seq free
    --PE--> V [2048,520] (seq on partitions; 65 cols/head: 64 V + ones)
  per (head-pair p, q-tile 512, k-tile 128):
    ST[k,q] via 2 row-tiled K=32 matmuls -> st psum [128, 1024]
    exp(ST/sqrt32) on ACT -> SBUF bf16   (softmax w/o max-sub: |scores|<=5.66)
    att[r] += V_aug[kt,h].T @ expST      (psum [65,512]; row 64 = sum_k exp = Z)
  normalize (pipelined across DVE/Pool; r=1 chain leads since attV
    accumulates r=1 first): 2x DVE reciprocal(Z) -> 2x GPSIMD
    partition_broadcast -> 2x DVE mul -> c_T. The odd head's mul writes
    ct rows 64-127 DIRECTLY (DVE accepts differing in/out partition
    bases, verified on hw) — the old SBUF partition-shift DMA and its
    ~2.7us queue+completion latency are gone.
  y = c_T.T @ WoT_g  (per-core partial, bf16 out; host upcasts + sums)

Scheduling (metronome + fill queue): DMA is split and ordered
(interleaved wke/xt-nt0 k-pairs, wqe, wv, xt-nt1..3, wo) so the first
encoder matmul starts ~1us in. Each unit (head-pair, qt) emits a tight
ST->exp stream — the metronome that keeps ACT saturated; between kt
quarters it drains NFILL pieces from a deque: the previous unit's
attV+normalize (queue front, one-unit lag decouples attV from exp),
then deferred prep from the back. Deferred encoder pieces (q0-nt2/3,
k1, q1-nt*) are staggered into the unit loop just before first use —
the back half (units 8-14) otherwise runs dry of fill work and the
1-kt exp lookahead stalls PE ~186ns per kt. Unit (0,0) interleaves the
quad-0 k-enc slices directly (hard dep of its kt sweep). PSUM: st
2x[128,1024] + att 2x[65,512] + bank 2x[128,512] = 8 banks. Tail: attV
of the last unit runs inline, remaining fills drain BEFORE its
normalize (tile-granular ct deps would otherwise serialize them after
the whole chain), then the staged out-proj (ready k<3 contractions
ahead of the normalize-gated last k) with psum->sbuf copies alternated
DVE/ACT and y DMAs alternated across the SP/ACT rings.

Probe-measured hw behavior (timing-only builds): halving attV matmuls
transfers ~1:1 to total time, halving ST only ~40%, halving exp ~22% —
per-kt PE work (ST+attV+fills ~1.45us) is the binding rate with ACT
(~1.14us/kt) close behind; both engines carry ~+40ns/instr of
LDW/sem overhead vs the cost model.

fp8e4 DoubleRow for the scores matmul was tried and removed:
microbenchmarks measured DR at parity/slower than bf16 on real silicon
(cost model claims 2x), and it costs 3.5x the rel-err margin. walrus
--enable-ldw-opt=true fails in visitInstLdweights (that's why the
compile path pins it false).
"""

import numpy as np
import ml_dtypes

import concourse.bacc as bacc
import concourse.bass as bass
import concourse.mybir as mybir
import concourse.tile as tile
from concourse.bass_utils import run_bass_kernel_spmd

B, N, D = 4, 2048, 1024
H, HD, MB = 16, 64, 32
NCORES = 8
HPG = 8              # heads per group (per core)
GD = HPG * HD        # 512 head dims per group
SCALE = float(1.0 / np.sqrt(MB))

bf16 = mybir.dt.bfloat16
f32 = mybir.dt.float32
BF = ml_dtypes.bfloat16
AF = mybir.ActivationFunctionType

KT_X = D // 128      # 8 contraction tiles over d_model
NT = N // 512        # 4 column tiles of 512 over sequence
MT_QK = GD // 128    # 4 partition tiles of QT/KT
NT128 = N // 128     # 16 row tiles of 128 over sequence
KT_C = GD // 128     # 4 contraction tiles over group head dims

TRACE = False        # set by test.py for profiling runs
TRACE_KW = {}
LAST_RESULTS = None
PHASE_LIMIT = "full"  # "qkv" | "attn" | "full" — for sim phase ablation
NFILL = 2            # fill pieces drained per metronome quarter


def build(reps=1):
    nc = bacc.Bacc("TRN2", target_bir_lowering=False, debug=False,
                   num_devices=NCORES)
    EB = HPG * MB        # 256 encoder bits per core (8 heads x 32)
    xt = nc.dram_tensor("xt", [D, N], bf16, kind="ExternalInput").ap()
    # folded encoder weights: Wqe = Wq_h^T @ W_enc[h] per head, [1024, 256]
    # (Q/K are consumed only through the encoders, so the full-width QK
    # projections fold away entirely)
    wqe = nc.dram_tensor("wqe", [D, EB], bf16, kind="ExternalInput").ap()
    wke = nc.dram_tensor("wke", [D, EB], bf16, kind="ExternalInput").ap()
    wv = nc.dram_tensor("wv", [D, GD], bf16, kind="ExternalInput").ap()
    wo = nc.dram_tensor("wo", [GD, D], bf16, kind="ExternalInput").ap()
    y = nc.dram_tensor("y", [N, D], bf16, kind="ExternalOutput").ap()

    with tile.TileContext(nc) as tc:
        with (
            tc.tile_pool(name="xtp", bufs=KT_X) as xt_pool,
            tc.tile_pool(name="wp", bufs=3 * KT_X) as w_pool,
            tc.tile_pool(name="wop", bufs=KT_C) as wo_pool,
            tc.tile_pool(name="encp", bufs=4) as enc_pool,
            tc.tile_pool(name="vp", bufs=NT128) as v_pool,
            tc.tile_pool(name="expp", bufs=16) as exp_pool,
            tc.tile_pool(name="ctp", bufs=KT_C) as ct_pool,
            tc.tile_pool(name="smallp", bufs=4) as small_pool,
            tc.tile_pool(name="yp", bufs=6) as y_pool,
            tc.tile_pool(name="stp", bufs=2, space="PSUM") as st_pool,
            tc.tile_pool(name="attp", bufs=2, space="PSUM") as att_pool,
            tc.tile_pool(name="bankp", bufs=2, space="PSUM") as bank_pool,
        ):
          for _rep in range(reps):
            # ---- input loads: each tensor's k-slices split across BOTH
            # HWDGE queues (even k -> SP, odd k -> ACT; ACT descriptors
            # precede every ACT compute instruction in program order), in
            # consumer-priority order so both rings fill the pipe in
            # parallel at ~2x the single-queue rate ---------------------------
            def load_w(w_ap, cols, name):
                tiles = []
                for k in range(KT_X):
                    t = w_pool.tile([128, cols], bf16, tag=f"w{cols}",
                                    name=f"{name}{k}")
                    nc.sync.dma_start(t[:], w_ap[128 * k:128 * (k + 1), :])
                    tiles.append(t)
                return tiles

            xt_sb = [xt_pool.tile([128, N], bf16, tag="xt", name=f"xt{k}")
                     for k in range(KT_X)]

            def load_xt_nt(nt):
                for k in range(KT_X):
                    nc.sync.dma_start(
                        xt_sb[k][:, 512 * nt:512 * (nt + 1)],
                        xt[128 * k:128 * (k + 1), 512 * nt:512 * (nt + 1)])

            # interleave wke k-tiles with xt nt0 slices so the first
            # encoder m-tile's k-loop starts after the first pair lands
            wke_sb = []
            for k in range(KT_X):
                t = w_pool.tile([128, EB], bf16, tag="w256", name=f"wke{k}")
                nc.sync.dma_start(t[:], wke[128 * k:128 * (k + 1), :])
                wke_sb.append(t)
                nc.sync.dma_start(
                    xt_sb[k][:, 0:512],
                    xt[128 * k:128 * (k + 1), 0:512])

            wqe_sb = load_w(wqe, EB, "wqe")
            wv_sb = load_w(wv, GD, "wv")
            for nt in range(1, NT):
                load_xt_nt(nt)
            wo_sb = []
            for k in range(KT_C):
                t = wo_pool.tile([128, D], bf16, tag="wo")
                nc.sync.dma_start(t[:], wo[128 * k:128 * (k + 1), :])
                wo_sb.append(t)

            # ---- encoders: [128 bits (4 heads x 32), 2048 seq] per quad ----
            q_enc = [enc_pool.tile([128, N], bf16, tag="enc",
                                   name=f"qenc{d}") for d in range(2)]
            k_enc = [enc_pool.tile([128, N], bf16, tag="enc",
                                   name=f"kenc{d}") for d in range(2)]

            def enc_mtile_nt(wsb, et, qd, nt, name):
                # one folded x->bits projection: psum[bit, seq] -> tanh
                ps = bank_pool.tile([128, 512], f32, tag="bank",
                                    name=f"ps_{name}_{nt}")
                for k in range(KT_X):
                    nc.tensor.matmul(
                        ps[:],
                        wsb[k][:, 128 * qd:128 * (qd + 1)],
                        xt_sb[k][:, 512 * nt:512 * (nt + 1)],
                        start=(k == 0), stop=(k == KT_X - 1),
                        skip_group_check=True,
                    )
                nc.scalar.activation(et[:, 512 * nt:512 * (nt + 1)],
                                     ps[:], AF.Tanh)

            v_sb = [None] * NT128

            def ensure_v(nt):
                if v_sb[nt] is not None:
                    return v_sb[nt]
                t = v_pool.tile([128, HPG * 65], bf16, tag="v", name=f"v{nt}")
                ps = bank_pool.tile([128, 512], f32, tag="bank",
                                    name=f"ps_v{nt}")
                for k in range(KT_X):
                    nc.tensor.matmul(
                        ps[:],
                        xt_sb[k][:, 128 * nt:128 * (nt + 1)],
                        wv_sb[k][:],
                        start=(k == 0), stop=(k == KT_X - 1),
                        skip_group_check=True,
                    )
                vv = t[:, :].rearrange("p (h s) -> p h s", h=HPG)
                nc.vector.tensor_copy(
                    vv[:, :, 0:64],
                    ps[:, :].rearrange("p (h s) -> p h s", h=HPG),
                )
                nc.vector.memset(vv[:, :, 64:65], 1.0)
                v_sb[nt] = t
                return t

            # ---- c_T accumulator tiles: [512 head dims, 2048 seq] -----------
            ct_sb = [ct_pool.tile([128, N], bf16, tag="ct", name=f"ct{i}")
                     for i in range(KT_C)]

            def st_exp_kt(p, qt, kt):
                """ST -> exp for heads (2p, 2p+1) at one kt; returns ex."""
                qd = p // 2
                st = st_pool.tile([128, N // 2], f32, tag="st")
                for r in range(2):
                    a = 2 * (p % 2) + r
                    nc.tensor.matmul(
                        st[:, 512 * r:512 * (r + 1)],
                        k_enc[qd][32 * a:32 * (a + 1),
                                  128 * kt:128 * (kt + 1)],
                        q_enc[qd][32 * a:32 * (a + 1),
                                  512 * qt:512 * (qt + 1)],
                        start=True, stop=True,
                        tile_position=(32 * a, 0),
                        skip_group_check=True,
                    )
                ex = exp_pool.tile([128, N // 2], bf16, tag="exp")
                nc.scalar.activation(ex[:], st[:], AF.Exp, scale=SCALE)
                return ex

            def attv_kts(p, att, exs, kt_lo, kt_hi):
                # r=1 first: normalize frees att[1] first (its chain is
                # emitted first), so the first accumulate here waits on the
                # earlier-freed tile
                for kt in range(kt_lo, kt_hi):
                    for r in (1, 0):
                        h = 2 * p + r
                        nc.tensor.matmul(
                            att[r][0:65, :],
                            ensure_v(kt)[:, 65 * h:65 * h + 65],
                            exs[kt][:, 512 * r:512 * (r + 1)],
                            start=(kt == 0), stop=(kt == NT128 - 1),
                            skip_group_check=True,
                        )

            def normalize(p, qt, att):
                # pipelined across DVE/Pool: both recips issue first, then
                # both broadcasts, then both muls; the r=1 chain leads and
                # everything is lane-aligned (r=1 data at partitions 64-127)
                # so ct is written directly — no partition-shift DMA
                # NOTE: reciprocal_approx_fast passes CoreSim but returns
                # garbage on hw through this compile path (custom-DVE
                # table likely not shipped by the bass2jax/axon NEFF
                # build) — keep the exact reciprocal
                qs = slice(512 * qt, 512 * (qt + 1))
                rec1 = small_pool.tile([1, 512], f32, tag="recip")
                nc.vector.reciprocal(rec1[:], att[1][64:65, :])
                rec0 = small_pool.tile([1, 512], f32, tag="recip")
                nc.vector.reciprocal(rec0[:], att[0][64:65, :])
                bc1 = small_pool.tile([64, 512], f32, tag="bc")
                nc.gpsimd.partition_broadcast(bc1[:], rec1[:])
                bc0 = small_pool.tile([64, 512], f32, tag="bc")
                nc.gpsimd.partition_broadcast(bc0[:], rec0[:])
                # odd head: DVE writes the partition-shifted destination
                # directly (in base 0 -> out base 64)
                nc.vector.tensor_mul(
                    ct_sb[p][64:128, qs], att[1][0:64, :], bc1[:])
                nc.vector.tensor_mul(
                    ct_sb[p][0:64, qs], att[0][0:64, :], bc0[:])

            def new_att(p, qt):
                # full-bank tiles: r=1 uses partitions 63-127, r=0 uses 0-64
                return [att_pool.tile([128, 512], f32, tag="att",
                                      name=f"att{p}_{qt}_{r}")
                        for r in range(2)]


            tail = [False]  # set before the i==15 fill drain

            def out_proj_mt(mt):
                # y rows 128*mt .. 128*(mt+1): 2 out-dim halves
                for nt2 in range(2):
                    ps = bank_pool.tile([128, 512], f32, tag="bank",
                                        name=f"ps_y{mt}_{nt2}")
                    for k in range(KT_C):
                        nc.tensor.matmul(
                            ps[:],
                            ct_sb[k][:, 128 * mt:128 * (mt + 1)],
                            wo_sb[k][:, 512 * nt2:512 * (nt2 + 1)],
                            start=(k == 0), stop=(k == KT_C - 1),
                            skip_group_check=True,
                        )
                    yt = y_pool.tile([128, 512], bf16, tag="y")
                    if tail[0]:
                        # tail-drained pieces: copy on the idle ACT engine
                        # so the normalize reciprocals aren't queued behind
                        # these copies in the DVE FIFO
                        nc.scalar.activation(yt[:], ps[:], AF.Copy)
                    else:
                        nc.vector.tensor_copy(yt[:], ps[:])
                    nc.sync.dma_start(
                        y[128 * mt:128 * (mt + 1),
                          512 * nt2:512 * (nt2 + 1)],
                        yt[:])

            def out_proj_qt(qt):
                for mt in range(4 * qt, 4 * qt + 4):
                    out_proj_mt(mt)

            def out_proj_qt_staged(qt):
                # tail variant: PE executes in order, so emit k=0..2 of two
                # groups before their k=3 (which waits on the last ct DMA);
                # the ready contractions fill the wait.
                slots = [(mt, nt2) for mt in range(4 * qt, 4 * qt + 4)
                         for nt2 in range(2)]
                for i in range(0, len(slots), 2):
                    pss = []
                    for mt, nt2 in slots[i:i + 2]:
                        ps = bank_pool.tile([128, 512], f32, tag="bank",
                                            name=f"ps_y{mt}_{nt2}")
                        for k in range(KT_C - 1):
                            nc.tensor.matmul(
                                ps[:],
                                ct_sb[k][:, 128 * mt:128 * (mt + 1)],
                                wo_sb[k][:, 512 * nt2:512 * (nt2 + 1)],
                                start=(k == 0), stop=False,
                                skip_group_check=True,
                            )
                        pss.append(ps)
                    for (mt, nt2), ps in zip(slots[i:i + 2], pss):
                        nc.tensor.matmul(
                            ps[:],
                            ct_sb[KT_C - 1][:, 128 * mt:128 * (mt + 1)],
                            wo_sb[KT_C - 1][:, 512 * nt2:512 * (nt2 + 1)],
                            start=False, stop=True,
                            skip_group_check=True,
                        )
                        yt = y_pool.tile([128, 512], bf16, tag="y")
                        # tail flush: alternate the psum->sbuf copies between
                        # DVE and ACT (both idle here) and the y DMAs between
                        # the SP/ACT rings, so neither engine serializes the
                        # last 8 tiles
                        if (mt + nt2) % 2:
                            nc.scalar.activation(yt[:], ps[:], AF.Copy)
                            nc.scalar.dma_start(
                                y[128 * mt:128 * (mt + 1),
                                  512 * nt2:512 * (nt2 + 1)],
                                yt[:])
                        else:
                            nc.vector.tensor_copy(yt[:], ps[:])
                            nc.sync.dma_start(
                                y[128 * mt:128 * (mt + 1),
                                  512 * nt2:512 * (nt2 + 1)],
                                yt[:])

            # ---- emission script -------------------------------------------
            # ramp-min: everything pair0-qt0 kt0-3 needs (k_enc cols 0-511,
            # q_enc cols 0-511, V0/V1), then interleave pair0-qt0's kt
            # quarters with the remaining k-enc nt slices.
            done_kq = set()

            def kq_piece(which, nt):
                # one deferred-prep piece: a folded-encoder quad-nt slice.
                # Idempotent: emitted by whichever of the queue or a unit
                # preamble reaches it first.
                if (which, nt) in done_kq:
                    return
                done_kq.add((which, nt))
                wsb, enc_dst, enc_name = {
                    "k0": (wke_sb, k_enc[0], "kenc0"),
                    "q0": (wqe_sb, q_enc[0], "qenc0"),
                    "k1": (wke_sb, k_enc[1], "kenc1"),
                    "q1": (wqe_sb, q_enc[1], "qenc1"),
                }[which]
                qd = int(which[1])
                enc_mtile_nt(wsb, enc_dst, qd, nt, enc_name)

            def need_enc(p, qt):
                # hard deps of unit (p, qt)'s kt sweep: its quad's k-enc in
                # full, plus the q-enc slice for this qt (no-ops if already
                # drained from the queue)
                qd = p // 2
                for nt in range(NT):
                    kq_piece(f"k{qd}", nt)
                kq_piece(f"q{qd}", qt)

            # ---- metronome + fill queue ------------------------------------
            # Each unit (pair, qt) emits a tight ST+exp stream (the metronome,
            # gating ACT); between kt quarters it drains fill pieces: the
            # previous unit's attV+normalize (front of queue), then deferred
            # prep (QK m-tiles/encoders, out-proj) from the back.
            from collections import deque
            fills = deque()

            def fill(n):
                for _ in range(n):
                    if fills:
                        fills.popleft()()

            def metronome(p, qt, nfill=2, direct=None):
                exs = []
                for q in range(NT):
                    for kt in range(4 * q, 4 * q + 4):
                        exs.append(st_exp_kt(p, qt, kt))
                    if direct is not None and q < NT - 1:
                        direct(q + 1)
                    fill(nfill)
                return exs

            def attv_norm_pieces(p, qt, att, exs):
                pieces = [
                    (lambda q=q: attv_kts(p, att, exs, 4 * q, 4 * q + 4))
                    for q in range(NT)
                ]
                pieces.append(lambda: normalize(p, qt, att))
                return pieces

            def out_piece(mt):
                return lambda: out_proj_mt(mt)

            # deferred prep, in first-use order, then V prefetch (fills
            # the otherwise-starved mid-kernel units; attV's inline
            # ensure_v makes any not-yet-drained piece a no-op)
            # only the pieces needed soon go in up front; the rest are
            # staggered into the back half of the unit loop (below), where
            # the fill queue otherwise runs dry and the metronome's 1-kt
            # exp lookahead stalls PE ~186ns per kt
            fills.append(lambda: kq_piece("q0", 1))
            for kt in range(2, NT128):
                fills.append(lambda kt=kt: (ensure_v(kt), None))
            deferred = {
                1: [lambda: kq_piece("q0", 2)],
                3: [lambda: kq_piece("q0", 3)] + [
                    lambda nt=nt: kq_piece("k1", nt) for nt in range(NT)],
                5: [lambda: kq_piece("q1", 0)],
                7: [lambda: kq_piece("q1", 1)],
                9: [lambda: kq_piece("q1", 2)],
                11: [lambda: kq_piece("q1", 3)],
            }

            # minimal ramp: quad-0 K and Q over nt0, V0/V1; unit (0,0)
            # interleaves the k0 nt>=1 slices directly (hard dep of its kt
            # sweep)
            kq_piece("k0", 0)
            kq_piece("q0", 0)
            ensure_v(0)
            ensure_v(1)

            order = [(0, 0), (1, 0), (0, 1), (1, 1), (0, 2), (1, 2),
                     (0, 3), (1, 3), (2, 0), (3, 0), (2, 1), (3, 1),
                     (2, 2), (3, 2), (2, 3), (3, 3)]
            for i, (p, qt) in enumerate(order):
                att = new_att(p, qt)
                if i == 0:
                    exs = metronome(p, qt, nfill=NFILL,
                                    direct=lambda nt: kq_piece("k0", nt))
                else:
                    need_enc(p, qt)
                    exs = metronome(p, qt, nfill=NFILL)
                # previous unit's attV/normalize already queued; queue ours
                # at the front so they run in the next unit's windows
                pieces = attv_norm_pieces(p, qt, att, exs)
                if i == len(order) - 1:
                    # tail: attV inline, then the remaining fills (their ct
                    # reads must precede normalize(15)'s ct writes — tile-
                    # granular deps would otherwise serialize them after the
                    # whole chain), then normalize, then the staged out-proj
                    for f in pieces[:-1]:
                        f()
                    tail[0] = True
                    while fills:
                        fills.popleft()()
                    pieces[-1]()
                else:
                    fills.extendleft(reversed(pieces))
                if p == 3 and qt < NT - 1:
                    # out-proj for qt becomes legal once pair3-qt normalize
                    # is queued; drains from the back of the queue
                    for mt in range(4 * qt, 4 * qt + 4):
                        fills.append(out_piece(mt))
                for f in deferred.get(i, ()):
                    fills.append(f)
                if PHASE_LIMIT == "qkv" and i == 0:
                    break
            if PHASE_LIMIT == "qkv":
                continue
            out_proj_qt_staged(NT - 1)
    nc.finalize()
    return nc


_nc_cache = None


def make_in_maps(inputs):
    x = np.asarray(inputs["x"], dtype=np.float32)
    Wq = np.asarray(inputs["Wq"], dtype=np.float32)
    Wk = np.asarray(inputs["Wk"], dtype=np.float32)
    Wv = np.asarray(inputs["Wv"], dtype=np.float32)
    We = np.asarray(inputs["W_enc"], dtype=np.float32)
    Wo = np.asarray(inputs["Wo"], dtype=np.float32)

    xts = [np.ascontiguousarray(x[b].T).astype(BF) for b in range(B)]
    in_maps = []
    for c in range(NCORES):
        b, g = divmod(c, 2)
        gs = g * GD
        # fold Q/K projections into the per-head bit encoders:
        # Wqe[:, 32i:32i+32] = Wq[head i rows].T @ W_enc[head i]
        # (Q/K are only ever consumed through tanh(Qh @ W_enc[h]))
        wqe = np.empty((D, HPG * MB), np.float32)
        wke = np.empty((D, HPG * MB), np.float32)
        for i in range(HPG):
            h = g * HPG + i
            wqe[:, MB * i:MB * (i + 1)] = \
                Wq[h * HD:(h + 1) * HD, :].T @ We[h]
            wke[:, MB * i:MB * (i + 1)] = \
                Wk[h * HD:(h + 1) * HD, :].T @ We[h]
        in_maps.append({
            "xt": xts[b],
            "wqe": wqe.astype(BF),
            "wke": wke.astype(BF),
            "wv": np.ascontiguousarray(Wv[gs:gs + GD, :].T).astype(BF),
            "wo": np.ascontiguousarray(Wo[:, gs:gs + GD].T).astype(BF),
        })
    return in_maps


def kernel(**inputs):
    global _nc_cache, LAST_RESULTS
    if _nc_cache is None:
        _nc_cache = build()
    nc = _nc_cache
    in_maps = make_in_maps(inputs)

    res = run_bass_kernel_spmd(
        nc, in_maps, core_ids=list(range(NCORES)),
        trace=TRACE, **TRACE_KW)
    LAST_RESULTS = res

    out = np.empty((B, N, D), dtype=np.float32)
    for b in range(B):
        out[b] = (res.results[2 * b]["y"].astype(np.float32)
                  + res.results[2 * b + 1]["y"].astype(np.float32))
    return out



# revision 39
# speedup vs baseline: 14.7375x; 1.0029x over previous
"""BREWA (bit-witness) attention on 8 TRN2 NeuronCores.

Sharding: core c = (batch b, head-group g) with b = c // 2, g = c % 2.
Each core computes its batch's attention for 8 of the 16 heads plus the
partial output projection over those heads' Wo columns; the host sums the
two partial projections per batch (the "all-reduce" is 2-way, done on CPU).

Key structural trick: Q and K are consumed ONLY through the per-head bit
encoders tanh((x@Wq^T)_h @ W_enc[h]), so the two linear maps are folded
on the host into Wqe[h] = Wq_h^T @ W_enc[h] ([1024 -> 32] per head) —
the full-width QK projections never exist on device (-35us PE, -224
matmuls, and one less bf16 rounding).

Per-core dataflow (all matmuls bf16, fp32 PSUM accumulate):
  xT[b] (host-transposed, bf16)
    --PE (folded Wqe/Wke, K=1024)--> enc psum --ACT tanh--> q_encT,k_encT
        [128, 2048] per quad: 4 heads x 32 bits on partitions, seq free
    --PE--> V [2048,520] (seq on partitions; 65 cols/head: 64 V + ones)
  per (head-pair p, q-tile 512, k-tile 128):
    ST[k,q] via 2 row-tiled K=32 matmuls -> st psum [128, 1024]
    exp(ST/sqrt32) on ACT -> SBUF bf16   (softmax w/o max-sub: |scores|<=5.66)
    att[r] += V_aug[kt,h].T @ expST      (psum [65,512]; row 64 = sum_k exp = Z)
  normalize (pipelined across DVE/Pool; r=1 chain leads since attV
    accumulates r=1 first): 2x DVE reciprocal(Z) -> 2x GPSIMD
    partition_broadcast -> 2x DVE mul -> c_T. The odd head's mul writes
    ct rows 64-127 DIRECTLY (DVE accepts differing in/out partition
    bases, verified on hw) — the old SBUF partition-shift DMA and its
    ~2.7us queue+completion latency are gone.
  y = c_T.T @ WoT_g  (per-core partial, bf16 out; host upcasts + sums)

Scheduling (metronome + fill queue): DMA is split and ordered
(interleaved wke/xt-nt0 k-pairs, wqe, wv, xt-nt1..3, wo) so the first
encoder matmul starts ~1us in. Each unit (head-pair, qt) emits a tight
ST->exp stream — the metronome that keeps ACT saturated; between kt
quarters it drains NFILL pieces from a deque: the previous unit's
attV+normalize (queue front, one-unit lag decouples attV from exp),
then deferred prep from the back. Deferred encoder pieces (q0-nt2/3,
k1, q1-nt*) are staggered into the unit loop just before first use —
the back half (units 8-14) otherwise runs dry of fill work and the
1-kt exp lookahead stalls PE ~186ns per kt. Unit (0,0) interleaves the
quad-0 k-enc slices directly (hard dep of its kt sweep). PSUM: st
2x[128,1024] + att 2x[65,512] + bank 2x[128,512] = 8 banks. Tail: attV
of the last unit runs inline, remaining fills drain BEFORE its
normalize (tile-granular ct deps would otherwise serialize them after
the whole chain), then the staged out-proj (ready k<3 contractions
ahead of the normalize-gated last k) with psum->sbuf copies alternated
DVE/ACT and y DMAs alternated across the SP/ACT rings.

Probe-measured hw behavior (timing-only builds): halving attV matmuls
transfers ~1:1 to total time, halving ST only ~40%, halving exp ~22% —
per-kt PE work (ST+attV+fills ~1.45us) is the binding rate with ACT
(~1.14us/kt) close behind; both engines carry ~+40ns/instr of
LDW/sem overhead vs the cost model.

fp8e4 DoubleRow for the scores matmul was tried and removed:
microbenchmarks measured DR at parity/slower than bf16 on real silicon
(cost model claims 2x), and it costs 3.5x the rel-err margin. walrus
--enable-ldw-opt=true fails in visitInstLdweights (that's why the
compile path pins it false).
"""

import numpy as np
import ml_dtypes

import concourse.bacc as bacc
import concourse.bass as bass
import concourse.mybir as mybir
import concourse.tile as tile
from concourse.bass_utils import run_bass_kernel_spmd

B, N, D = 4, 2048, 1024
H, HD, MB = 16, 64, 32
NCORES = 8
HPG = 8              # heads per group (per core)
GD = HPG * HD        # 512 head dims per group
SCALE = float(1.0 / np.sqrt(MB))

bf16 = mybir.dt.bfloat16
f32 = mybir.dt.float32
BF = ml_dtypes.bfloat16
AF = mybir.ActivationFunctionType

KT_X = D // 128      # 8 contraction tiles over d_model
NT = N // 512        # 4 column tiles of 512 over sequence
MT_QK = GD // 128    # 4 partition tiles of QT/KT
NT128 = N // 128     # 16 row tiles of 128 over sequence
KT_C = GD // 128     # 4 contraction tiles over group head dims

TRACE = False        # set by test.py for profiling runs
TRACE_KW = {}
LAST_RESULTS = None
PHASE_LIMIT = "full"  # "qkv" | "attn" | "full" — for sim phase ablation
NFILL = 2            # fill pieces drained per metronome quarter


def build(reps=1):
    nc = bacc.Bacc("TRN2", target_bir_lowering=False, debug=False,
                   num_devices=NCORES)
    EB = HPG * MB        # 256 encoder bits per core (8 heads x 32)
    xt = nc.dram_tensor("xt", [D, N], bf16, kind="ExternalInput").ap()
    # folded encoder weights: Wqe = Wq_h^T @ W_enc[h] per head, [1024, 256]
    # (Q/K are consumed only through the encoders, so the full-width QK
    # projections fold away entirely)
    wqe = nc.dram_tensor("wqe", [D, EB], bf16, kind="ExternalInput").ap()
    # ramp = host-packed [wke | xt nt0] so the start-critical pair arrives
    # as ONE descriptor per k-tile instead of two (halves the DMA pacing
    # of the first encoder m-tile's k-loop)
    ramp = nc.dram_tensor("ramp", [D, EB + 512], bf16,
                          kind="ExternalInput").ap()
    wv = nc.dram_tensor("wv", [D, GD], bf16, kind="ExternalInput").ap()
    wo = nc.dram_tensor("wo", [GD, D], bf16, kind="ExternalInput").ap()
    y = nc.dram_tensor("y", [N, D], bf16, kind="ExternalOutput").ap()

    with tile.TileContext(nc) as tc:
        with (
            tc.tile_pool(name="xtp", bufs=KT_X) as xt_pool,
            tc.tile_pool(name="wp", bufs=3 * KT_X) as w_pool,
            tc.tile_pool(name="rampp", bufs=KT_X) as ramp_pool,
            tc.tile_pool(name="wop", bufs=KT_C) as wo_pool,
            tc.tile_pool(name="encp", bufs=4) as enc_pool,
            tc.tile_pool(name="vp", bufs=NT128) as v_pool,
            tc.tile_pool(name="expp", bufs=16) as exp_pool,
            tc.tile_pool(name="ctp", bufs=KT_C) as ct_pool,
            tc.tile_pool(name="smallp", bufs=4) as small_pool,
            tc.tile_pool(name="yp", bufs=6) as y_pool,
            tc.tile_pool(name="stp", bufs=2, space="PSUM") as st_pool,
            tc.tile_pool(name="attp", bufs=2, space="PSUM") as att_pool,
            tc.tile_pool(name="bankp", bufs=2, space="PSUM") as bank_pool,
        ):
          for _rep in range(reps):
            # ---- input loads: each tensor's k-slices split across BOTH
            # HWDGE queues (even k -> SP, odd k -> ACT; ACT descriptors
            # precede every ACT compute instruction in program order), in
            # consumer-priority order so both rings fill the pipe in
            # parallel at ~2x the single-queue rate ---------------------------
            def load_w(w_ap, cols, name):
                tiles = []
                for k in range(KT_X):
                    t = w_pool.tile([128, cols], bf16, tag=f"w{cols}",
                                    name=f"{name}{k}")
                    nc.sync.dma_start(t[:], w_ap[128 * k:128 * (k + 1), :])
                    tiles.append(t)
                return tiles

            # xt cols 0-511 live in the ramp tiles; xt_sb holds cols 512+
            xt_sb = [xt_pool.tile([128, N - 512], bf16, tag="xt",
                                  name=f"xt{k}") for k in range(KT_X)]

            def load_xt_nt(nt):
                for k in range(KT_X):
                    nc.sync.dma_start(
                        xt_sb[k][:, 512 * (nt - 1):512 * nt],
                        xt[128 * k:128 * (k + 1), 512 * nt:512 * (nt + 1)])

            # one merged descriptor per k-tile carries wke k AND xt nt0 k
            wke_sb = []
            for k in range(KT_X):
                t = ramp_pool.tile([128, EB + 512], bf16, tag="ramp",
                                   name=f"ramp{k}")
                nc.sync.dma_start(t[:], ramp[128 * k:128 * (k + 1), :])
                wke_sb.append(t)

            def xt_cols(k, lo, hi):
                # xt columns 0-511 live in the ramp tile (offset EB)
                if hi <= 512:
                    return wke_sb[k][:, EB + lo:EB + hi]
                return xt_sb[k][:, lo - 512:hi - 512]

            wqe_sb = load_w(wqe, EB, "wqe")
            wv_sb = load_w(wv, GD, "wv")
            for nt in range(1, NT):
                load_xt_nt(nt)
            wo_sb = []
            for k in range(KT_C):
                t = wo_pool.tile([128, D], bf16, tag="wo")
                nc.sync.dma_start(t[:], wo[128 * k:128 * (k + 1), :])
                wo_sb.append(t)

            # ---- encoders: [128 bits (4 heads x 32), 2048 seq] per quad ----
            q_enc = [enc_pool.tile([128, N], bf16, tag="enc",
                                   name=f"qenc{d}") for d in range(2)]
            k_enc = [enc_pool.tile([128, N], bf16, tag="enc",
                                   name=f"kenc{d}") for d in range(2)]

            def enc_mtile_nt(wsb, et, qd, nt, name):
                # one folded x->bits projection: psum[bit, seq] -> tanh
                ps = bank_pool.tile([128, 512], f32, tag="bank",
                                    name=f"ps_{name}_{nt}")
                for k in range(KT_X):
                    nc.tensor.matmul(
                        ps[:],
                        wsb[k][:, 128 * qd:128 * (qd + 1)],
                        xt_cols(k, 512 * nt, 512 * (nt + 1)),
                        start=(k == 0), stop=(k == KT_X - 1),
                        skip_group_check=True,
                    )
                nc.scalar.activation(et[:, 512 * nt:512 * (nt + 1)],
                                     ps[:], AF.Tanh)

            v_sb = [None] * NT128

            def ensure_v(nt):
                if v_sb[nt] is not None:
                    return v_sb[nt]
                t = v_pool.tile([128, HPG * 65], bf16, tag="v", name=f"v{nt}")
                ps = bank_pool.tile([128, 512], f32, tag="bank",
                                    name=f"ps_v{nt}")
                for k in range(KT_X):
                    nc.tensor.matmul(
                        ps[:],
                        xt_cols(k, 128 * nt, 128 * (nt + 1)),
                        wv_sb[k][:],
                        start=(k == 0), stop=(k == KT_X - 1),
                        skip_group_check=True,
                    )
                vv = t[:, :].rearrange("p (h s) -> p h s", h=HPG)
                nc.vector.tensor_copy(
                    vv[:, :, 0:64],
                    ps[:, :].rearrange("p (h s) -> p h s", h=HPG),
                )
                nc.vector.memset(vv[:, :, 64:65], 1.0)
                v_sb[nt] = t
                return t

            # ---- c_T accumulator tiles: [512 head dims, 2048 seq] -----------
            ct_sb = [ct_pool.tile([128, N], bf16, tag="ct", name=f"ct{i}")
                     for i in range(KT_C)]

            def st_exp_kt(p, qt, kt):
                """ST -> exp for heads (2p, 2p+1) at one kt; returns ex."""
                qd = p // 2
                st = st_pool.tile([128, N // 2], f32, tag="st")
                for r in range(2):
                    a = 2 * (p % 2) + r
                    nc.tensor.matmul(
                        st[:, 512 * r:512 * (r + 1)],
                        k_enc[qd][32 * a:32 * (a + 1),
                                  128 * kt:128 * (kt + 1)],
                        q_enc[qd][32 * a:32 * (a + 1),
                                  512 * qt:512 * (qt + 1)],
                        start=True, stop=True,
                        tile_position=(32 * a, 0),
                        skip_group_check=True,
                    )
                ex = exp_pool.tile([128, N // 2], bf16, tag="exp")
                nc.scalar.activation(ex[:], st[:], AF.Exp, scale=SCALE)
                return ex

            def attv_kts(p, att, exs, kt_lo, kt_hi):
                # r=1 first: normalize frees att[1] first (its chain is
                # emitted first), so the first accumulate here waits on the
                # earlier-freed tile
                for kt in range(kt_lo, kt_hi):
                    for r in (1, 0):
                        h = 2 * p + r
                        nc.tensor.matmul(
                            att[r][0:65, :],
                            ensure_v(kt)[:, 65 * h:65 * h + 65],
                            exs[kt][:, 512 * r:512 * (r + 1)],
                            start=(kt == 0), stop=(kt == NT128 - 1),
                            skip_group_check=True,
                        )

            def normalize(p, qt, att):
                # pipelined across DVE/Pool: both recips issue first, then
                # both broadcasts, then both muls; the r=1 chain leads and
                # everything is lane-aligned (r=1 data at partitions 64-127)
                # so ct is written directly — no partition-shift DMA
                # NOTE: reciprocal_approx_fast passes CoreSim but returns
                # garbage on hw through this compile path (custom-DVE
                # table likely not shipped by the bass2jax/axon NEFF
                # build) — keep the exact reciprocal
                qs = slice(512 * qt, 512 * (qt + 1))
                rec1 = small_pool.tile([1, 512], f32, tag="recip")
                nc.vector.reciprocal(rec1[:], att[1][64:65, :])
                rec0 = small_pool.tile([1, 512], f32, tag="recip")
                nc.vector.reciprocal(rec0[:], att[0][64:65, :])
                bc1 = small_pool.tile([64, 512], f32, tag="bc")
                nc.gpsimd.partition_broadcast(bc1[:], rec1[:])
                bc0 = small_pool.tile([64, 512], f32, tag="bc")
                nc.gpsimd.partition_broadcast(bc0[:], rec0[:])
                # odd head: DVE writes the partition-shifted destination
                # directly (in base 0 -> out base 64)
                nc.vector.tensor_mul(
                    ct_sb[p][64:128, qs], att[1][0:64, :], bc1[:])
                nc.vector.tensor_mul(
                    ct_sb[p][0:64, qs], att[0][0:64, :], bc0[:])

            def new_att(p, qt):
                # full-bank tiles: r=1 uses partitions 63-127, r=0 uses 0-64
                return [att_pool.tile([128, 512], f32, tag="att",
                                      name=f"att{p}_{qt}_{r}")
                        for r in range(2)]


            tail = [False]  # set before the i==15 fill drain

            def out_proj_mt(mt):
                # y rows 128*mt .. 128*(mt+1): 2 out-dim halves
                for nt2 in range(2):
                    ps = bank_pool.tile([128, 512], f32, tag="bank",
                                        name=f"ps_y{mt}_{nt2}")
                    for k in range(KT_C):
                        nc.tensor.matmul(
                            ps[:],
                            ct_sb[k][:, 128 * mt:128 * (mt + 1)],
                            wo_sb[k][:, 512 * nt2:512 * (nt2 + 1)],
                            start=(k == 0), stop=(k == KT_C - 1),
                            skip_group_check=True,
                        )
                    yt = y_pool.tile([128, 512], bf16, tag="y")
                    if tail[0]:
                        # tail-drained pieces: copy on the idle ACT engine
                        # so the normalize reciprocals aren't queued behind
                        # these copies in the DVE FIFO
                        nc.scalar.activation(yt[:], ps[:], AF.Copy)
                    else:
                        nc.vector.tensor_copy(yt[:], ps[:])
                    nc.sync.dma_start(
                        y[128 * mt:128 * (mt + 1),
                          512 * nt2:512 * (nt2 + 1)],
                        yt[:])

            def out_proj_qt(qt):
                for mt in range(4 * qt, 4 * qt + 4):
                    out_proj_mt(mt)

            def out_proj_qt_staged(qt):
                # tail variant: PE executes in order, so emit k=0..2 of two
                # groups before their k=3 (which waits on the last ct DMA);
                # the ready contractions fill the wait.
                slots = [(mt, nt2) for mt in range(4 * qt, 4 * qt + 4)
                         for nt2 in range(2)]
                for i in range(0, len(slots), 2):
                    pss = []
                    for mt, nt2 in slots[i:i + 2]:
                        ps = bank_pool.tile([128, 512], f32, tag="bank",
                                            name=f"ps_y{mt}_{nt2}")
                        for k in range(KT_C - 1):
                            nc.tensor.matmul(
                                ps[:],
                                ct_sb[k][:, 128 * mt:128 * (mt + 1)],
                                wo_sb[k][:, 512 * nt2:512 * (nt2 + 1)],
                                start=(k == 0), stop=False,
                                skip_group_check=True,
                            )
                        pss.append(ps)
                    for (mt, nt2), ps in zip(slots[i:i + 2], pss):
                        nc.tensor.matmul(
                            ps[:],
                            ct_sb[KT_C - 1][:, 128 * mt:128 * (mt + 1)],
                            wo_sb[KT_C - 1][:, 512 * nt2:512 * (nt2 + 1)],
                            start=False, stop=True,
                            skip_group_check=True,
                        )
                        yt = y_pool.tile([128, 512], bf16, tag="y")
                        # tail flush: alternate the psum->sbuf copies between
                        # DVE and ACT (both idle here) and the y DMAs between
                        # the SP/ACT rings, so neither engine serializes the
                        # last 8 tiles
                        if (mt + nt2) % 2:
                            nc.scalar.activation(yt[:], ps[:], AF.Copy)
                            nc.scalar.dma_start(
                                y[128 * mt:128 * (mt + 1),
                                  512 * nt2:512 * (nt2 + 1)],
                                yt[:])
                        else:
                            nc.vector.tensor_copy(yt[:], ps[:])
                            nc.sync.dma_start(
                                y[128 * mt:128 * (mt + 1),
                                  512 * nt2:512 * (nt2 + 1)],
                                yt[:])

            # ---- emission script -------------------------------------------
            # ramp-min: everything pair0-qt0 kt0-3 needs (k_enc cols 0-511,
            # q_enc cols 0-511, V0/V1), then interleave pair0-qt0's kt
            # quarters with the remaining k-enc nt slices.
            done_kq = set()

            def kq_piece(which, nt):
                # one deferred-prep piece: a folded-encoder quad-nt slice.
                # Idempotent: emitted by whichever of the queue or a unit
                # preamble reaches it first.
                if (which, nt) in done_kq:
                    return
                done_kq.add((which, nt))
                wsb, enc_dst, enc_name = {
                    "k0": (wke_sb, k_enc[0], "kenc0"),
                    "q0": (wqe_sb, q_enc[0], "qenc0"),
                    "k1": (wke_sb, k_enc[1], "kenc1"),
                    "q1": (wqe_sb, q_enc[1], "qenc1"),
                }[which]
                qd = int(which[1])
                enc_mtile_nt(wsb, enc_dst, qd, nt, enc_name)

            def need_enc(p, qt):
                # hard deps of unit (p, qt)'s kt sweep: its quad's k-enc in
                # full, plus the q-enc slice for this qt (no-ops if already
                # drained from the queue)
                qd = p // 2
                for nt in range(NT):
                    kq_piece(f"k{qd}", nt)
                kq_piece(f"q{qd}", qt)

            # ---- metronome + fill queue ------------------------------------
            # Each unit (pair, qt) emits a tight ST+exp stream (the metronome,
            # gating ACT); between kt quarters it drains fill pieces: the
            # previous unit's attV+normalize (front of queue), then deferred
            # prep (QK m-tiles/encoders, out-proj) from the back.
            from collections import deque
            fills = deque()

            def fill(n):
                for _ in range(n):
                    if fills:
                        fills.popleft()()

            def metronome(p, qt, nfill=2, direct=None):
                exs = []
                for q in range(NT):
                    for kt in range(4 * q, 4 * q + 4):
                        exs.append(st_exp_kt(p, qt, kt))
                    if direct is not None and q < NT - 1:
                        direct(q + 1)
                    fill(nfill)
                return exs

            def attv_norm_pieces(p, qt, att, exs):
                pieces = [
                    (lambda q=q: attv_kts(p, att, exs, 4 * q, 4 * q + 4))
                    for q in range(NT)
                ]
                pieces.append(lambda: normalize(p, qt, att))
                return pieces

            def out_piece(mt):
                return lambda: out_proj_mt(mt)

            # deferred prep, in first-use order, then V prefetch (fills
            # the otherwise-starved mid-kernel units; attV's inline
            # ensure_v makes any not-yet-drained piece a no-op)
            # only the pieces needed soon go in up front; the rest are
            # staggered into the back half of the unit loop (below), where
            # the fill queue otherwise runs dry and the metronome's 1-kt
            # exp lookahead stalls PE ~186ns per kt
            fills.append(lambda: kq_piece("q0", 1))
            for kt in range(2, NT128):
                fills.append(lambda kt=kt: (ensure_v(kt), None))
            deferred = {
                1: [lambda: kq_piece("q0", 2)],
                3: [lambda: kq_piece("q0", 3)] + [
                    lambda nt=nt: kq_piece("k1", nt) for nt in range(NT)],
                5: [lambda: kq_piece("q1", 0)],
                7: [lambda: kq_piece("q1", 1)],
                9: [lambda: kq_piece("q1", 2)],
                11: [lambda: kq_piece("q1", 3)],
            }

            # minimal ramp: quad-0 K and Q over nt0, V0/V1; unit (0,0)
            # interleaves the k0 nt>=1 slices directly (hard dep of its kt
            # sweep)
            kq_piece("k0", 0)
            kq_piece("q0", 0)
            ensure_v(0)
            ensure_v(1)

            order = [(0, 0), (1, 0), (0, 1), (1, 1), (0, 2), (1, 2),
                     (0, 3), (1, 3), (2, 0), (3, 0), (2, 1), (3, 1),
                     (2, 2), (3, 2), (2, 3), (3, 3)]
            for i, (p, qt) in enumerate(order):
                att = new_att(p, qt)
                if i == 0:
                    exs = metronome(p, qt, nfill=NFILL,
                                    direct=lambda nt: kq_piece("k0", nt))
                else:
                    need_enc(p, qt)
                    exs = metronome(p, qt, nfill=NFILL)
                # previous unit's attV/normalize already queued; queue ours
                # at the front so they run in the next unit's windows
                pieces = attv_norm_pieces(p, qt, att, exs)
                if i == len(order) - 1:
                    # tail: attV inline, then the remaining fills (their ct
                    # reads must precede normalize(15)'s ct writes — tile-
                    # granular deps would otherwise serialize them after the
                    # whole chain), then normalize, then the staged out-proj
                    for f in pieces[:-1]:
                        f()
                    tail[0] = True
                    while fills:
                        fills.popleft()()
                    pieces[-1]()
                else:
                    fills.extendleft(reversed(pieces))
                if p == 3 and qt < NT - 1:
                    # out-proj for qt becomes legal once pair3-qt normalize
                    # is queued; drains from the back of the queue
                    for mt in range(4 * qt, 4 * qt + 4):
                        fills.append(out_piece(mt))
                for f in deferred.get(i, ()):
                    fills.append(f)
                if PHASE_LIMIT == "qkv" and i == 0:
                    break
            if PHASE_LIMIT == "qkv":
                continue
            out_proj_qt_staged(NT - 1)
    nc.finalize()
    return nc


_nc_cache = None


def make_in_maps(inputs):
    x = np.asarray(inputs["x"], dtype=np.float32)
    Wq = np.asarray(inputs["Wq"], dtype=np.float32)
    Wk = np.asarray(inputs["Wk"], dtype=np.float32)
    Wv = np.asarray(inputs["Wv"], dtype=np.float32)
    We = np.asarray(inputs["W_enc"], dtype=np.float32)
    Wo = np.asarray(inputs["Wo"], dtype=np.float32)

    xts = [np.ascontiguousarray(x[b].T).astype(BF) for b in range(B)]
    in_maps = []
    for c in range(NCORES):
        b, g = divmod(c, 2)
        gs = g * GD
        # fold Q/K projections into the per-head bit encoders:
        # Wqe[:, 32i:32i+32] = Wq[head i rows].T @ W_enc[head i]
        # (Q/K are only ever consumed through tanh(Qh @ W_enc[h]))
        wqe = np.empty((D, HPG * MB), np.float32)
        wke = np.empty((D, HPG * MB), np.float32)
        for i in range(HPG):
            h = g * HPG + i
            wqe[:, MB * i:MB * (i + 1)] = \
                Wq[h * HD:(h + 1) * HD, :].T @ We[h]
            wke[:, MB * i:MB * (i + 1)] = \
                Wk[h * HD:(h + 1) * HD, :].T @ We[h]
        # ramp: [wke | xt nt0] packed so each 128-row k-tile is one
        # contiguous DMA descriptor on device
        ramp = np.concatenate(
            [wke.astype(BF), xts[b][:, :512]], axis=1)
        in_maps.append({
            "xt": xts[b],
            "wqe": wqe.astype(BF),
            "ramp": np.ascontiguousarray(ramp),
            "wv": np.ascontiguousarray(Wv[gs:gs + GD, :].T).astype(BF),
            "wo": np.ascontiguousarray(Wo[:, gs:gs + GD].T).astype(BF),
        })
    return in_maps


def kernel(**inputs):
    global _nc_cache, LAST_RESULTS
    if _nc_cache is None:
        _nc_cache = build()
    nc = _nc_cache
    in_maps = make_in_maps(inputs)

    res = run_bass_kernel_spmd(
        nc, in_maps, core_ids=list(range(NCORES)),
        trace=TRACE, **TRACE_KW)
    LAST_RESULTS = res

    out = np.empty((B, N, D), dtype=np.float32)
    for b in range(B):
        out[b] = (res.results[2 * b]["y"].astype(np.float32)
                  + res.results[2 * b + 1]["y"].astype(np.float32))
    return out

